# revision 13
# baseline (speedup 1.0000x reference)
"""GATv2 (3 live layers) + sum-pooling + MLP head on 8 Trainium2 NeuronCores.

Sharding: nodes/edges sharded by destination-node range across 8 cores.
Per layer (all tables bf16): sharded dense phase (xl/xr projections, BN folded
into weights), AllGather of the xl table, edge phase with two large batched
dma_gathers per 64-chunk supergroup (src rows from the global table, dst rows
from the local table), scores via scalar-engine LeakyReLU + vector mult/reduce,
softmax weights folded into the gathered payload, selection-matrix bf16 matmul
aggregation into aligned 128-node windows (denominator via a ones column),
PE-accumulated global pools/stats, fp32 MLP head replicated on every core.

Self-contained: hardcodes shapes from the problem spec; host side does only
integer index preprocessing and O(d^2) weight folds.
"""
import os
import sys

sys.path.insert(0, "/opt/trn_rl_repo")

import ml_dtypes
import numpy as np

import concourse.bass as bass
import concourse.bacc as bacc
import concourse.tile as tile
from concourse import mybir
from concourse import bass_utils
from concourse.library_config import mlp as _mlp_lib
from concourse.masks import make_identity

F32 = mybir.dt.float32
BF16 = mybir.dt.float16
I16 = mybir.dt.int16
OP = mybir.AluOpType
AF = mybir.ActivationFunctionType
BFNP = np.float16

N, E, GRAPHS = 50000, 400000, 256
NDEV = 8
MID = 32768
PAD_SHIFT = 60000.0
BN_EPS = 1e-5
SG_CHUNKS = 8  # chunks per gather supergroup (8*128 = 1024 indices)


class Cfg:
    def __init__(self, n=N, e=E, g=GRAPHS, ndev=NDEV):
        assert n % ndev == 0
        self.n, self.e, self.g, self.ndev = n, e, g, ndev
        self.n_loc = n // ndev
        self.n_win = (self.n_loc + 127) // 128
        self.n_pad = self.n_win * 128
        self.n_tab = self.n_pad * ndev
        self.cw = None
        self.base = MID if self.n_tab > 32000 else 0
        self.iw = max(g, 128)
        self.dims_in = [128, 128, 64]
        self.dims_out = [128, 64, 32]
        self.tw = 128  # table width (bf16 rows must be 256B-aligned)


def preprocess(edge_index, batch, cfg: Cfg):
    n, ndev, n_loc, n_win = cfg.n, cfg.ndev, cfg.n_loc, cfg.n_win
    src = np.concatenate([np.asarray(edge_index[0]), np.arange(n)]).astype(np.int64)
    dst = np.concatenate([np.asarray(edge_index[1]), np.arange(n)]).astype(np.int64)
    batch = np.asarray(batch).astype(np.int64)

    def pad_id(x):
        return (x // n_loc) * (n_win * 128) + (x % n_loc)

    dev_of = dst // n_loc
    dev_data = []
    max_cw = 1
    for d in range(ndev):
        m = dev_of == d
        s_d, t_d = src[m], dst[m] - d * n_loc
        padn = np.arange(n_loc, n_win * 128)
        s_d = np.concatenate([s_d, np.full(len(padn), d * n_loc)])
        t_d = np.concatenate([t_d, padn])
        order = np.argsort(t_d, kind="stable")
        s_d, t_d = s_d[order], t_d[order]
        cnts = np.bincount(t_d // 128, minlength=n_win)
        max_cw = max(max_cw, int(np.ceil(cnts.max() / 128)))
        dev_data.append((s_d, t_d, cnts))

    cfg.cw = cw = max_cw
    n_chunks = n_win * cw
    n_chunks_pad = ((n_chunks + 7) // 8) * 8
    L = n_chunks_pad * 128

    per_dev = []
    for d in range(ndev):
        s_d, t_d, cnts = dev_data[d]
        slot_src = np.full(L, cfg.base, dtype=np.int64)
        slot_rdst = np.zeros(L, dtype=np.int64)
        slot_shift = np.full(L, PAD_SHIFT, dtype=np.float32)
        pos = 0
        for w in range(n_win):
            cnt = int(cnts[w])
            base = w * cw * 128
            sl = slice(pos, pos + cnt)
            slot_src[base:base + cnt] = pad_id(s_d[sl])
            slot_rdst[base:base + cnt] = t_d[sl]
            slot_shift[base:base + cnt] = (t_d[sl] - w * 128).astype(np.float32)
            pos += cnt
        assert pos == len(s_d)

        # each 1024-slot gather block must end with a non-negative (src-base)
        # index: trailing negative int16 idxs are dropped by the gather ucode.
        if cfg.base > 0:
            for gb in range(0, L, 1024):
                if slot_src[gb + 1023] - cfg.base < 0:
                    cand = np.where(slot_src[gb:gb + 1024] - cfg.base >= 0)[0]
                    assert len(cand), "gather group has no non-negative index"
                    j = gb + cand[-1]
                    for arr in (slot_src, slot_rdst, slot_shift):
                        arr[j], arr[gb + 1023] = arr[gb + 1023], arr[j]

        def wrap16(vals):
            v = vals.astype(np.int16)
            return np.tile(v.reshape(-1, 16).T, (8, 1)).copy()

        loc_nodes = np.arange(n_win * 128)
        glob_nodes = np.minimum(d * n_loc + loc_nodes, n - 1)
        bglob = batch[glob_nodes]
        valid = loc_nodes < n_loc
        # static per-window graph-selection matrix [128, n_win, G+1]:
        # selg[p, w, j] = 1 if node (w,p) valid and in graph j; col G = valid
        selg = np.zeros((128, n_win, cfg.g + 1), dtype=BFNP)
        bg2 = bglob.reshape(n_win, 128).T
        vd2 = valid.reshape(n_win, 128).T
        for w in range(n_win):
            for p in range(128):
                if vd2[p, w]:
                    selg[p, w, bg2[p, w]] = 1.0
                    selg[p, w, cfg.g] = 1.0

        per_dev.append(dict(
            src16=wrap16(slot_src - cfg.base),
            rdst16=wrap16(slot_rdst),
            dst_shift=slot_shift.reshape(n_chunks_pad, 128).T.astype(BFNP).copy(),
            selg=selg,
        ))
    return per_dev, n_chunks_pad


def build_program(cfg: Cfg, n_chunks_pad: int, scratch=16384):
    ndev, n_win, cw = cfg.ndev, cfg.n_win, cfg.cw
    dims_in, dims_out = cfg.dims_in, cfg.dims_out
    NL = n_win * 128
    K = n_chunks_pad
    n_layers = len(dims_in)
    Gn = cfg.g
    TW = cfg.tw

    nc = bacc.Bacc("TRN2", target_bir_lowering=False, debug=False,
                   enable_asserts=False, num_devices=ndev,
                   dynamic_dma_scratch_size=scratch, num_swdge_queues=1)

    def din(name, shape, dt=F32):
        return nc.dram_tensor(name, shape, dt, kind="ExternalInput").ap()

    xt_in = din("xt_in", [128, NL], BF16)
    src16_in = din("src16_in", [128, K * 8], I16)
    rdst16_in = din("rdst16_in", [128, K * 8], I16)
    dshift_in = din("dshift_in", [128, K], BF16)
    selg_in = din("selg_in", [128, n_win, Gn + 1], BF16)
    cntrep_in = din("cntrep_in", [128, Gn])
    iota8_in = din("iota8_in", [128, 8 * 128], BF16)
    onesrow_in = din("onesrow_in", [1, 128])
    w_in, arep_in, blc_in, g_in, be_in = {}, {}, {}, {}, {}
    for i in range(1, n_layers + 1):
        di, do = dims_in[i - 1], dims_out[i - 1]
        w_in[i] = din(f"wcat{i}_in", [di, 2 * do], BF16 if i == 1 else F32)
        arep_in[i] = din(f"arep{i}_in", [128, TW], BF16)
        blc_in[i] = din(f"blc{i}_in", [1, 2 * do])
        g_in[i] = din(f"g{i}_in", [do, 1])
        be_in[i] = din(f"be{i}_in", [do, 1])
    w5_in = din("w5_in", [224, 128])
    b5_in = din("b5_in", [128, 1])
    g5_in = din("g5_in", [128, 1])
    be5_in = din("be5_in", [128, 1])
    w6_in = din("w6_in", [128, 10])
    b6_in = din("b6_in", [10, 1])

    out_dram = nc.dram_tensor("out", [2, Gn, 10], F32, kind="ExternalOutput").ap()

    nc.gpsimd.load_library(_mlp_lib)

    with tile.TileContext(nc) as tc:
        with tc.tile_pool(name="const", bufs=1) as cst, \
             tc.tile_pool(name="persist", bufs=1) as per, \
             tc.tile_pool(name="dram", bufs=1, space="DRAM") as dram:

            iota8 = cst.tile([128, 8, 128], BF16)
            nc.sync.dma_start(iota8[:].rearrange("p a b -> p (a b)"), iota8_in[:])
            onesrow = cst.tile([1, 128], F32)
            nc.sync.dma_start(onesrow[:], onesrow_in[:])
            ident = cst.tile([128, 128], F32)
            make_identity(nc, ident[:])
            identb = cst.tile([128, 128], BF16)
            make_identity(nc, identb[:])
            epscol = cst.tile([128, 1], F32)
            nc.vector.memset(epscol[:], BN_EPS)
            xt = per.tile([128, NL], BF16)
            nc.sync.dma_start(xt[:], xt_in[:])
            src16 = per.tile([128, K * 8], I16)
            nc.sync.dma_start(src16[:], src16_in[:])
            rdst16 = per.tile([128, K * 8], I16)
            nc.sync.dma_start(rdst16[:], rdst16_in[:])
            dshift = per.tile([128, K], BF16)
            nc.sync.dma_start(dshift[:], dshift_in[:])
            selg_all = per.tile([128, n_win, Gn + 1], BF16)
            nc.sync.dma_start(
                selg_all[:].rearrange("p a b -> p (a b)"),
                selg_in[:].rearrange("p a b -> p (a b)"))

            ybuf = {i: per.tile([128, n_win, dims_out[i - 1]], BF16,
                                name=f"ybuf{i}")
                    for i in range(1, n_layers + 1)}
            arep = {}
            for i in range(1, n_layers + 1):
                arep[i] = per.tile([128, TW], BF16, name=f"arep{i}")
                nc.sync.dma_start(arep[i][:], arep_in[i][:])
            pool_sb = {i: per.tile([dims_out[i - 1], Gn + 1], F32, name=f"pool{i}")
                       for i in range(1, n_layers + 1)}
            s2_sb = {i: per.tile([dims_out[i - 1], 1], F32, name=f"s2_{i}")
                     for i in range(1, n_layers + 1)}
            alpha = {i: per.tile([dims_out[i - 1], 1], F32, name=f"alpha{i}")
                     for i in range(1, n_layers + 1)}
            beta = {i: per.tile([dims_out[i - 1], 1], F32, name=f"beta{i}")
                    for i in range(1, n_layers + 1)}

            xl_dram, xltab_dram, st_dram, st_shared, xr_dram = {}, {}, {}, {}, {}
            for i in range(1, n_layers + 1):
                do = dims_out[i - 1]
                xl_dram[i] = dram.tile([NL, TW], BF16, name=f"xld{i}")
                xltab_dram[i] = dram.tile([cfg.n_tab, TW], BF16,
                                          addr_space="Shared", name=f"xltab{i}")
                xr_dram[i] = dram.tile([NL, TW], BF16, name=f"xrd{i}")
                st_dram[i] = dram.tile([do, 2], F32, name=f"std{i}")
                st_shared[i] = dram.tile([do, 2], F32, addr_space="Shared",
                                         name=f"sts{i}")
            hcat_dram = dram.tile([224, Gn], F32, name="hcatd")
            hcat_shared = dram.tile([224, Gn], F32, addr_space="Shared",
                                    name="hcats")
            rg = [list(range(ndev))]

            sg_counter = [0]
            for li in range(1, n_layers + 1):
                di, do = dims_in[li - 1], dims_out[li - 1]
                _dense_phase(nc, tc, cfg, li, di, do, xt, ybuf, w_in,
                             blc_in, alpha, beta, xl_dram[li], xr_dram[li],
                             identb, onesrow, n_win)
                nc.gpsimd.collective_compute(
                    "AllGather", OP.bypass, replica_groups=rg,
                    ins=[xl_dram[li][:]], outs=[xltab_dram[li][:]])
                _edge_phase(nc, tc, cfg, li, do, K, n_win, cw,
                            xltab_dram[li], xr_dram[li], src16, rdst16, dshift,
                            arep[li], iota8, ybuf[li], pool_sb[li], s2_sb[li],
                            selg_all, sg_counter)
                nc.sync.dma_start(st_dram[li][:, 0:1], pool_sb[li][:, Gn:Gn + 1])
                nc.sync.dma_start(st_dram[li][:, 1:2], s2_sb[li][:])
                nc.gpsimd.collective_compute(
                    "AllReduce", OP.add, replica_groups=rg,
                    ins=[st_dram[li][:]], outs=[st_shared[li][:]])
                _bn_coeffs(nc, tc, cfg, li, do, st_shared[li], g_in[li],
                           be_in[li], alpha[li], beta[li], epscol)

            _head(nc, tc, cfg, pool_sb, alpha, beta, cntrep_in, hcat_dram,
                  hcat_shared, w5_in, b5_in, g5_in, be5_in, w6_in, b6_in,
                  onesrow, ident, out_dram, rg, epscol)

    nc.compile()
    return nc


def _dense_phase(nc, tc, cfg, li, di, do, xt, ybuf, w_in, blc_in,
                 alpha, beta, xl_d, xr_d, identb, onesrow, n_win):
    TW = cfg.tw
    with tc.tile_pool(name=f"dn{li}", bufs=3) as sb, \
         tc.tile_pool(name=f"dnp{li}", bufs=2, space="PSUM") as ps, \
         tc.tile_pool(name=f"dnw{li}", bufs=1) as wp:
        wcat = wp.tile([di, 2 * do], BF16)
        bias_rep = wp.tile([128, 2 * do], BF16)
        brow = wp.tile([1, 2 * do], F32)
        blc = wp.tile([1, 2 * do], F32)
        nc.sync.dma_start(blc[:], blc_in[li][:])
        if li == 1:
            nc.sync.dma_start(wcat[:], w_in[1][:])
            nc.vector.tensor_copy(brow[:], blc[:])
        else:
            wraw = wp.tile([di, 2 * do], F32)
            nc.sync.dma_start(wraw[:], w_in[li][:])
            nc.vector.tensor_scalar(out=wcat[:], in0=wraw[:],
                                    scalar1=alpha[li - 1][:], scalar2=None,
                                    op0=OP.mult)
            brow_ps = ps.tile([1, 2 * do], F32, space="PSUM", tag="brow", bufs=1)
            nc.tensor.matmul(brow_ps[:], lhsT=beta[li - 1][:], rhs=wraw[:],
                             start=True, stop=True)
            nc.vector.tensor_tensor(out=brow[:], in0=brow_ps[:], in1=blc[:],
                                    op=OP.add)
        bias_ps = ps.tile([128, 2 * do], F32, space="PSUM", tag="bias", bufs=1)
        nc.tensor.matmul(bias_ps[:], lhsT=onesrow[:], rhs=brow[:],
                         start=True, stop=True)
        nc.vector.tensor_copy(bias_rep[:], bias_ps[:])

        xlb = wp.tile([128, n_win, TW], BF16)
        xrb = wp.tile([128, n_win, TW], BF16)
        if TW > do:
            nc.vector.memset(xlb[:, :, do:], 0.0)
            nc.vector.memset(xrb[:, :, do:], 0.0)
        for w in range(n_win):
            if li == 1:
                lhs = xt[:, w * 128:(w + 1) * 128]
            else:
                tr_ps = ps.tile([di, 128], BF16, space="PSUM", tag="tr")
                nc.tensor.transpose(out=tr_ps[:], in_=ybuf[li - 1][:, w, :],
                                    identity=identb[:])
                tr = sb.tile([di, 128], BF16, tag="tr_sb")
                nc.scalar.activation(tr[:], tr_ps[:], AF.Copy)
                lhs = tr[:]
            o_ps = ps.tile([128, 2 * do], F32, space="PSUM", tag="o")
            nc.tensor.matmul(o_ps[:], lhsT=lhs, rhs=wcat[:], start=True,
                             stop=True)
            nc.vector.tensor_tensor(out=xlb[:, w, :do], in0=o_ps[:, :do],
                                    in1=bias_rep[:, :do], op=OP.add)
            nc.vector.tensor_tensor(out=xrb[:, w, :do], in0=o_ps[:, do:],
                                    in1=bias_rep[:, do:], op=OP.add)
        nc.sync.dma_start(xl_d[:].rearrange("(w p) d -> p w d", p=128), xlb[:])
        nc.sync.dma_start(xr_d[:].rearrange("(w p) d -> p w d", p=128), xrb[:])


def _edge_phase(nc, tc, cfg, li, do, K, n_win, cw, xltab, xr_tab,
                src16, rdst16, dshift, arep_l, iota8, ybuf_l, pool_l, s2_l,
                selg_all, sg_counter):
    n_chunks = n_win * cw
    Gn = cfg.g
    TW = cfg.tw
    EW = do + 1  # aggregated width: payload + softmax-denominator column
    with tc.tile_pool(name=f"eg{li}", bufs=2) as gb, \
         tc.tile_pool(name=f"et{li}", bufs=2) as tb, \
         tc.tile_pool(name=f"es{li}", bufs=3) as eb, \
         tc.tile_pool(name=f"ea{li}", bufs=2, space="PSUM") as aps, \
         tc.tile_pool(name=f"epp{li}", bufs=1, space="PSUM") as pps:
        pool_ps = pps.tile([do, Gn + 1], F32, space="PSUM", name=f"poolps{li}")
        s2_ps = pps.tile([do, 1], F32, space="PSUM", name=f"s2ps{li}")

        win_psums = {}
        for c0 in range(0, K, SG_CHUNKS):
            c1 = min(c0 + SG_CHUNKS, K)
            nsg = c1 - c0
            ni = nsg * 128
            sgi = sg_counter[0]
            sg_counter[0] += 1
            mbuf = gb.tile([128, nsg, TW], BF16, tag="m")
            nc.gpsimd.dma_gather(mbuf[:], xltab[cfg.base:, :],
                                 src16[:, c0 * 8:c1 * 8], ni, ni, TW,
                                 queue_num=0)
            rbuf = gb.tile([128, nsg, TW], BF16, tag="r")
            nc.gpsimd.dma_gather(rbuf[:], xr_tab[:],
                                 rdst16[:, c0 * 8:c1 * 8], ni, ni, TW,
                                 queue_num=0)
            # scores: e = a . lrelu(xl[src] + xr[dst])
            nc.vector.tensor_tensor(out=rbuf[:, :, :do], in0=mbuf[:, :, :do],
                                    in1=rbuf[:, :, :do], op=OP.add)
            uu = eb.tile([128, nsg, do], BF16, tag="uu")
            nc.vector.scalar_tensor_tensor(
                out=uu[:], in0=rbuf[:, :, :do], scalar=0.2,
                in1=rbuf[:, :, :do], op0=OP.mult, op1=OP.max)
            az = eb.tile([128, nsg, do], BF16, tag="az")
            nc.vector.tensor_tensor(
                out=az[:], in0=uu[:],
                in1=arep_l[:, None, :do].to_broadcast([128, nsg, do]),
                op=OP.mult)
            hd = do // 2
            azh = eb.tile([128, nsg, hd], BF16, tag="azh")
            nc.vector.tensor_tensor(out=azh[:], in0=az[:, :, :hd],
                                    in1=az[:, :, hd:do], op=OP.add)
            sg = eb.tile([128, nsg], F32, tag="sg")
            nc.vector.tensor_reduce(out=sg[:], in_=azh[:],
                                    axis=mybir.AxisListType.X, op=OP.add)
            pg = eb.tile([128, nsg], BF16, tag="pg")
            nc.scalar.activation(pg[:], sg[:], AF.Exp)
            # softmax-weighted payload (+ weight itself in column `do`)
            tts = tb.tile([128, nsg, EW], BF16, tag="tts")
            if do < TW:
                nc.vector.memset(mbuf[:, :, do:do + 1], 1.0)
                nc.vector.scalar_tensor_tensor(
                    out=tts[:], in0=mbuf[:, :, :EW], scalar=1.0,
                    in1=pg[:, :, None].to_broadcast([128, nsg, EW]),
                    op0=OP.mult, op1=OP.mult)
            else:
                nc.vector.scalar_tensor_tensor(
                    out=tts[:, :, :do], in0=mbuf[:], scalar=1.0,
                    in1=pg[:, :, None].to_broadcast([128, nsg, do]),
                    op0=OP.mult, op1=OP.mult)
                nc.scalar.activation(tts[:, :, do:do + 1], pg[:, :, None],
                                     AF.Copy)
            sel0 = tb.tile([128, nsg, 128], BF16, tag="sel0")
            nc.vector.scalar_tensor_tensor(
                out=sel0[:], in0=iota8[:, :nsg, :], scalar=1.0,
                in1=dshift[:, c0:c1, None].to_broadcast([128, nsg, 128]),
                op0=OP.mult, op1=OP.is_equal)
            for c in range(nsg):
                k = c0 + c
                if k >= n_chunks:
                    break
                w = k // cw
                if k % cw == 0:
                    win_psums[w] = aps.tile([128, EW], F32, space="PSUM",
                                            tag="agg", name=f"agg{li}_{w}")
                first, last = (k % cw == 0), (k % cw == cw - 1)
                nc.tensor.matmul(win_psums[w][:], lhsT=sel0[:, c, :],
                                 rhs=tts[:, c, :], start=first, stop=last)
                if last:
                    _window_epilogue(nc, eb, cfg, do, w, win_psums.pop(w),
                                     ybuf_l, selg_all, pool_ps, s2_ps,
                                     n_win, Gn)
        nc.vector.tensor_copy(pool_l[:], pool_ps[:])
        nc.vector.tensor_copy(s2_l[:], s2_ps[:])


def _window_epilogue(nc, eb, cfg, do, w, agg_ps, ybuf_l, selg_all,
                     pool_ps, s2_ps, n_win, Gn):
    recip = eb.tile([128, 1], F32, tag="recip")
    nc.vector.reciprocal_approx_fast(recip[:], agg_ps[:, do:do + 1])
    nc.scalar.activation(ybuf_l[:, w, :], agg_ps[:, :do], AF.Relu,
                         scale=recip[:])
    y2 = eb.tile([128, do], BF16, tag="y2")
    nc.scalar.activation(y2[:], ybuf_l[:, w, :], AF.Square)
    nc.tensor.matmul(pool_ps[:], lhsT=ybuf_l[:, w, :], rhs=selg_all[:, w, :],
                     start=(w == 0), stop=(w == n_win - 1))
    nc.tensor.matmul(s2_ps[:], lhsT=y2[:], rhs=selg_all[:, w, Gn:Gn + 1],
                     start=(w == 0), stop=(w == n_win - 1))


def _bn_coeffs(nc, tc, cfg, li, do, st_shared, g_in, be_in, alpha_l, beta_l,
               epscol):
    inv_n = 1.0 / cfg.n
    with tc.tile_pool(name=f"bn{li}", bufs=1) as sb:
        st = sb.tile([do, 2], F32)
        nc.sync.dma_start(st[:], st_shared[:])
        gc = sb.tile([do, 1], F32)
        nc.sync.dma_start(gc[:], g_in[:])
        bec = sb.tile([do, 1], F32)
        nc.sync.dma_start(bec[:], be_in[:])
        mean = sb.tile([do, 1], F32)
        nc.vector.tensor_scalar(out=mean[:], in0=st[:, 0:1], scalar1=inv_n,
                                scalar2=None, op0=OP.mult)
        msq = sb.tile([do, 1], F32)
        nc.vector.tensor_tensor(out=msq[:], in0=mean[:], in1=mean[:], op=OP.mult)
        var = sb.tile([do, 1], F32)
        nc.vector.scalar_tensor_tensor(out=var[:], in0=st[:, 1:2], scalar=inv_n,
                                       in1=msq[:], op0=OP.mult, op1=OP.subtract)
        sd = sb.tile([do, 1], F32)
        nc.scalar.activation(sd[:], var[:], AF.Sqrt, bias=epscol[:do, :])
        rsd = sb.tile([do, 1], F32)
        nc.vector.reciprocal(rsd[:], sd[:])
        nc.vector.tensor_tensor(out=alpha_l[:], in0=gc[:], in1=rsd[:], op=OP.mult)
        ma = sb.tile([do, 1], F32)
        nc.vector.tensor_tensor(out=ma[:], in0=mean[:], in1=alpha_l[:],
                                op=OP.mult)
        nc.vector.tensor_tensor(out=beta_l[:], in0=bec[:], in1=ma[:],
                                op=OP.subtract)


def _head(nc, tc, cfg, pool_sb, alpha, beta, cntrep_in, hcat_dram, hcat_shared,
          w5_in, b5_in, g5_in, be5_in, w6_in, b6_in, onesrow, ident, out_dram,
          rg, epscol):
    Gn = cfg.g
    dims_out = cfg.dims_out
    row_off = [0, 128, 192]
    with tc.tile_pool(name="hd", bufs=1) as sb, \
         tc.tile_pool(name="hdp", bufs=2, space="PSUM") as ps:
        cnt_rep = sb.tile([128, Gn], F32)
        nc.sync.dma_start(cnt_rep[:], cntrep_in[:])
        for i in range(1, len(dims_out) + 1):
            do = dims_out[i - 1]
            pf = sb.tile([do, Gn], F32, name=f"pf{i}")
            nc.vector.tensor_scalar(out=pf[:], in0=pool_sb[i][:, :Gn],
                                    scalar1=alpha[i][:], scalar2=None,
                                    op0=OP.mult)
            nc.vector.scalar_tensor_tensor(out=pf[:], in0=cnt_rep[:do, :],
                                           scalar=beta[i][:], in1=pf[:],
                                           op0=OP.mult, op1=OP.add)
            nc.sync.dma_start(hcat_dram[row_off[i - 1]:row_off[i - 1] + do, :],
                              pf[:])
        nc.gpsimd.collective_compute(
            "AllReduce", OP.add, replica_groups=rg,
            ins=[hcat_dram[:]], outs=[hcat_shared[:]])

        hc_top = sb.tile([128, Gn], F32)
        nc.sync.dma_start(hc_top[:], hcat_shared[0:128, :])
        hc_bot = sb.tile([96, Gn], F32)
        nc.sync.dma_start(hc_bot[:], hcat_shared[128:224, :])
        w5a = sb.tile([128, 128], F32)
        nc.sync.dma_start(w5a[:], w5_in[0:128, :])
        w5b = sb.tile([96, 128], F32)
        nc.sync.dma_start(w5b[:], w5_in[128:224, :])
        b5 = sb.tile([128, 1], F32)
        nc.sync.dma_start(b5[:], b5_in[:])
        h5_ps = ps.tile([128, Gn], F32, space="PSUM", tag="h5")
        nc.tensor.matmul(h5_ps[:], lhsT=w5a[:], rhs=hc_top[:], start=True,
                         stop=False)
        nc.tensor.matmul(h5_ps[:], lhsT=w5b[:], rhs=hc_bot[:], start=False,
                         stop=True)
        h5 = sb.tile([128, Gn], F32)
        nc.scalar.activation(h5[:], h5_ps[:], AF.Relu, bias=b5[:])
        # BN over the graph axis (free): biased var, eps
        scr = sb.tile([128, Gn], F32)
        s1 = sb.tile([128, 1], F32)
        nc.scalar.activation(scr[:], h5[:], AF.Copy, accum_out=s1[:])
        s2 = sb.tile([128, 1], F32)
        nc.scalar.activation(scr[:], h5[:], AF.Square, accum_out=s2[:])
        inv_g = 1.0 / Gn
        mean = sb.tile([128, 1], F32)
        nc.vector.tensor_scalar(out=mean[:], in0=s1[:], scalar1=inv_g,
                                scalar2=None, op0=OP.mult)
        msq = sb.tile([128, 1], F32)
        nc.vector.tensor_tensor(out=msq[:], in0=mean[:], in1=mean[:], op=OP.mult)
        var = sb.tile([128, 1], F32)
        nc.vector.scalar_tensor_tensor(out=var[:], in0=s2[:], scalar=inv_g,
                                       in1=msq[:], op0=OP.mult, op1=OP.subtract)
        sd = sb.tile([128, 1], F32)
        nc.scalar.activation(sd[:], var[:], AF.Sqrt, bias=epscol[:])
        rsd = sb.tile([128, 1], F32)
        nc.vector.reciprocal(rsd[:], sd[:])
        g5 = sb.tile([128, 1], F32)
        nc.sync.dma_start(g5[:], g5_in[:])
        be5 = sb.tile([128, 1], F32)
        nc.sync.dma_start(be5[:], be5_in[:])
        a5 = sb.tile([128, 1], F32)
        nc.vector.tensor_tensor(out=a5[:], in0=g5[:], in1=rsd[:], op=OP.mult)
        ma = sb.tile([128, 1], F32)
        nc.vector.tensor_tensor(out=ma[:], in0=mean[:], in1=a5[:], op=OP.mult)
        b5n = sb.tile([128, 1], F32)
        nc.vector.tensor_tensor(out=b5n[:], in0=be5[:], in1=ma[:], op=OP.subtract)
        h5n = sb.tile([128, Gn], F32)
        nc.scalar.activation(h5n[:], h5[:], AF.Identity, scale=a5[:], bias=b5n[:])

        w6 = sb.tile([128, 10], F32)
        nc.sync.dma_start(w6[:], w6_in[:])
        b6 = sb.tile([10, 1], F32)
        nc.sync.dma_start(b6[:], b6_in[:])
        lg_ps = ps.tile([10, Gn], F32, space="PSUM", tag="lg")
        nc.tensor.matmul(lg_ps[:], lhsT=w6[:], rhs=h5n[:], start=True, stop=True)
        lg = sb.tile([10, Gn], F32)
        nc.scalar.activation(lg[:], lg_ps[:], AF.Identity, bias=b6[:])

        nblk = Gn // 128 if Gn >= 128 else 1
        blk = min(128, Gn)
        lgn = sb.tile([128, nblk, 10], F32)
        for b in range(nblk):
            t_ps = ps.tile([blk, 10], F32, space="PSUM", tag="tr")
            nc.tensor.transpose(out=t_ps[:], in_=lg[:, b * blk:(b + 1) * blk],
                                identity=ident[:10, :10])
            nc.vector.tensor_copy(lgn[:blk, b, :], t_ps[:])
        sig = sb.tile([128, nblk, 10], F32)
        nc.scalar.activation(sig[:blk], lgn[:blk], AF.Sigmoid)
        mx = sb.tile([128, nblk], F32)
        nc.vector.tensor_reduce(out=mx[:blk], in_=lgn[:blk],
                                axis=mybir.AxisListType.X, op=OP.max)
        dd = sb.tile([128, nblk, 10], F32)
        nc.vector.tensor_tensor(out=dd[:blk], in0=lgn[:blk],
                                in1=mx[:blk, :, None].to_broadcast([blk, nblk, 10]),
                                op=OP.subtract)
        ee = sb.tile([128, nblk, 10], F32)
        nc.scalar.activation(ee[:blk], dd[:blk], AF.Exp)
        ssum = sb.tile([128, nblk], F32)
        nc.vector.tensor_reduce(out=ssum[:blk], in_=ee[:blk],
                                axis=mybir.AxisListType.X, op=OP.add)
        lns = sb.tile([128, nblk], F32)
        nc.scalar.activation(lns[:blk], ssum[:blk], AF.Ln)
        lsm = sb.tile([128, nblk, 10], F32)
        nc.vector.tensor_tensor(out=lsm[:blk], in0=dd[:blk],
                                in1=lns[:blk, :, None].to_broadcast([blk, nblk, 10]),
                                op=OP.subtract)
        nc.sync.dma_start(
            out_dram[0, :, :].rearrange("(w p) c -> p w c", p=blk), sig[:blk])
        nc.sync.dma_start(
            out_dram[1, :, :].rearrange("(w p) c -> p w c", p=blk), lsm[:blk])


# ---------------- host-side input packing & runner ----------------

def _fold_weights(inputs, cfg: Cfg):
    """Host-side static folds (small O(d^2) numpy)."""
    f = {}
    for i in range(1, 4):
        do = cfg.dims_out[i - 1]
        wl = np.asarray(inputs[f"Wl{i}"], np.float32)
        wr = np.asarray(inputs[f"Wr{i}"], np.float32)
        wcat = np.concatenate([wl, wr], axis=1)
        f[f"wcat{i}_in"] = wcat.astype(BFNP) if i == 1 else wcat
        a = np.asarray(inputs[f"a{i}"], np.float32)
        arep = np.zeros((128, cfg.tw), np.float32)
        arep[:, :do] = a[None, :]
        f[f"arep{i}_in"] = arep.astype(BFNP)
        bl = np.asarray(inputs[f"bl{i}"], np.float32)
        br = np.asarray(inputs[f"br{i}"], np.float32)
        bc = np.asarray(inputs[f"bc{i}"], np.float32)
        f[f"blc{i}_in"] = np.concatenate([bl + bc, br - bc])[None, :]
        f[f"g{i}_in"] = np.asarray(inputs[f"g{i}"], np.float32)[:, None]
        f[f"be{i}_in"] = np.asarray(inputs[f"be{i}"], np.float32)[:, None]
    w5 = np.asarray(inputs["W5"], np.float32)
    w5eff = w5[:224].copy()
    w5eff[192:224] += w5[224:256]
    f["w5_in"] = w5eff
    f["b5_in"] = np.asarray(inputs["b5"], np.float32)[:, None]
    f["g5_in"] = np.asarray(inputs["g5"], np.float32)[:, None]
    f["be5_in"] = np.asarray(inputs["be5"], np.float32)[:, None]
    f["w6_in"] = np.asarray(inputs["W6"], np.float32)
    f["b6_in"] = np.asarray(inputs["b6"], np.float32)[:, None]
    return f


def build_in_maps(inputs, cfg: Cfg, per_dev):
    x = np.asarray(inputs["x"], np.float32)
    batch = np.asarray(inputs["batch"], np.int64)
    folds = _fold_weights(inputs, cfg)
    cnt = np.bincount(batch, minlength=cfg.g).astype(np.float32)
    cnt_rep = np.tile(cnt[None, :], (128, 1))
    iota8 = np.tile(np.arange(128, dtype=np.float32), (128, 8)).astype(BFNP)
    onesrow = np.ones((1, 128), np.float32)
    in_maps = []
    zeros_cnt = np.zeros_like(cnt_rep)
    for d in range(cfg.ndev):
        pd = per_dev[d]
        xs = np.zeros((cfg.n_pad, 128), np.float32)
        xs[:cfg.n_loc] = x[d * cfg.n_loc:(d + 1) * cfg.n_loc]
        xt = xs.T.astype(BFNP).copy()
        im = dict(xt_in=xt, src16_in=pd["src16"], rdst16_in=pd["rdst16"],
                  dshift_in=pd["dst_shift"], selg_in=pd["selg"],
                  cntrep_in=cnt_rep if d == 0 else zeros_cnt, iota8_in=iota8,
                  onesrow_in=onesrow, **folds)
        in_maps.append(im)
    return in_maps


_CACHE = {}


def _get_program(cfg: Cfg, n_chunks_pad: int):
    key = (cfg.n, cfg.e, cfg.g, cfg.ndev, cfg.cw, n_chunks_pad)
    if key not in _CACHE:
        _CACHE[key] = build_program(cfg, n_chunks_pad)
    return _CACHE[key]


def _maybe_profile():
    """Optional NTFF capture driven by GAT_PROFILE_DIR (self-contained)."""
    import contextlib
    d = os.environ.get("GAT_PROFILE_DIR")
    if not d:
        return contextlib.nullcontext()
    import ctypes
    import glob

    os.makedirs(d, exist_ok=True)
    for f in glob.glob(d + "/*"):
        os.remove(f)
    lib = ctypes.CDLL("/opt/axon/libaxon_pjrt.so")
    lib.axon_start_nrt_profile.argtypes = [ctypes.POINTER(ctypes.c_int64),
                                           ctypes.c_size_t]
    lib.axon_start_nrt_profile.restype = ctypes.c_int64
    lib.axon_stop_nrt_profile.argtypes = [ctypes.c_char_p]
    lib.axon_stop_nrt_profile.restype = ctypes.c_int64

    @contextlib.contextmanager
    def ctx():
        import jax
        jax.devices()
        rc = lib.axon_start_nrt_profile(None, 0)
        if rc != 0:
            raise RuntimeError(f"profile start rc={rc}")
        try:
            yield
        finally:
            lib.axon_stop_nrt_profile(str(d).encode())

    return ctx()


def kernel(**inputs):
    cfg = Cfg()
    per_dev, n_chunks_pad = preprocess(inputs["edge_index"], inputs["batch"], cfg)
    nc = _get_program(cfg, n_chunks_pad)
    in_maps = build_in_maps(inputs, cfg, per_dev)
    with _maybe_profile():
        res = bass_utils.run_bass_kernel_spmd(nc, in_maps,
                                              core_ids=list(range(cfg.ndev)))
    out = np.asarray(res.results[0]["out"])
    return (out[0], out[1])


# revision 17
# speedup vs baseline: 1.7353x; 1.7353x over previous
"""GATv2 (3 live layers) + sum-pooling + MLP head on 8 Trainium2 NeuronCores.

Sharding: nodes/edges sharded by destination-node range across 8 cores.
Per layer (all tables bf16): sharded dense phase (xl/xr projections, BN folded
into weights), AllGather of the xl table, edge phase with two large batched
dma_gathers per 64-chunk supergroup (src rows from the global table, dst rows
from the local table), scores via scalar-engine LeakyReLU + vector mult/reduce,
softmax weights folded into the gathered payload, selection-matrix bf16 matmul
aggregation into aligned 128-node windows (denominator via a ones column),
PE-accumulated global pools/stats, fp32 MLP head replicated on every core.

Self-contained: hardcodes shapes from the problem spec; host side does only
integer index preprocessing and O(d^2) weight folds.
"""
import os
import sys

sys.path.insert(0, "/opt/trn_rl_repo")

import ml_dtypes
import numpy as np

import concourse.bass as bass
import concourse.bacc as bacc
import concourse.tile as tile
from concourse import mybir
from concourse import bass_utils
from concourse.library_config import mlp as _mlp_lib
from concourse.masks import make_identity

F32 = mybir.dt.float32
BF16 = mybir.dt.float16
I16 = mybir.dt.int16
OP = mybir.AluOpType
AF = mybir.ActivationFunctionType
BFNP = np.float16

N, E, GRAPHS = 50000, 400000, 256
NDEV = 8
MID = 32768
PAD_SHIFT = 60000.0
BN_EPS = 1e-5
SG_CHUNKS = 8  # chunks per gather supergroup (8*128 = 1024 indices)
SIM_MODE = False  # simtest sets True: single SWDGE queue to satisfy CoreSim's lane check


class Cfg:
    def __init__(self, n=N, e=E, g=GRAPHS, ndev=NDEV):
        assert n % ndev == 0
        self.n, self.e, self.g, self.ndev = n, e, g, ndev
        self.n_loc = n // ndev
        self.n_win = (self.n_loc + 127) // 128
        self.n_pad = self.n_win * 128
        self.n_tab = self.n_pad * ndev
        self.cw = None
        self.base = MID if self.n_tab > 32000 else 0
        self.iw = max(g, 128)
        self.dims_in = [128, 128, 64]
        self.dims_out = [128, 64, 32]
        self.tw = 128  # table width (bf16 rows must be 256B-aligned)


def preprocess(edge_index, batch, cfg: Cfg):
    n, ndev, n_loc, n_win = cfg.n, cfg.ndev, cfg.n_loc, cfg.n_win
    src = np.concatenate([np.asarray(edge_index[0]), np.arange(n)]).astype(np.int64)
    dst = np.concatenate([np.asarray(edge_index[1]), np.arange(n)]).astype(np.int64)
    batch = np.asarray(batch).astype(np.int64)

    def pad_id(x):
        return (x // n_loc) * (n_win * 128) + (x % n_loc)

    dev_of = dst // n_loc
    dev_data = []
    max_cw = 1
    for d in range(ndev):
        m = dev_of == d
        s_d, t_d = src[m], dst[m] - d * n_loc
        padn = np.arange(n_loc, n_win * 128)
        s_d = np.concatenate([s_d, np.full(len(padn), d * n_loc)])
        t_d = np.concatenate([t_d, padn])
        order = np.argsort(t_d, kind="stable")
        s_d, t_d = s_d[order], t_d[order]
        cnts = np.bincount(t_d // 128, minlength=n_win)
        max_cw = max(max_cw, int(np.ceil(cnts.max() / 128)))
        dev_data.append((s_d, t_d, cnts))

    cfg.cw = cw = max_cw
    n_chunks = n_win * cw
    n_chunks_pad = ((n_chunks + 7) // 8) * 8
    L = n_chunks_pad * 128

    per_dev = []
    for d in range(ndev):
        s_d, t_d, cnts = dev_data[d]
        slot_src = np.full(L, cfg.base, dtype=np.int64)
        slot_rdst = np.zeros(L, dtype=np.int64)
        slot_shift = np.full(L, PAD_SHIFT, dtype=np.float32)
        pos = 0
        for w in range(n_win):
            cnt = int(cnts[w])
            base = w * cw * 128
            sl = slice(pos, pos + cnt)
            slot_src[base:base + cnt] = pad_id(s_d[sl])
            slot_rdst[base:base + cnt] = t_d[sl]
            slot_shift[base:base + cnt] = (t_d[sl] - w * 128).astype(np.float32)
            pos += cnt
        assert pos == len(s_d)

        # each 1024-slot gather block must end with a non-negative (src-base)
        # index: trailing negative int16 idxs are dropped by the gather ucode.
        if cfg.base > 0:
            for gb in range(0, L, 1024):
                if slot_src[gb + 1023] - cfg.base < 0:
                    cand = np.where(slot_src[gb:gb + 1024] - cfg.base >= 0)[0]
                    assert len(cand), "gather group has no non-negative index"
                    j = gb + cand[-1]
                    for arr in (slot_src, slot_rdst, slot_shift):
                        arr[j], arr[gb + 1023] = arr[gb + 1023], arr[j]

        def wrap16(vals):
            v = vals.astype(np.int16)
            return np.tile(v.reshape(-1, 16).T, (8, 1)).copy()

        loc_nodes = np.arange(n_win * 128)
        glob_nodes = np.minimum(d * n_loc + loc_nodes, n - 1)
        bglob = batch[glob_nodes]
        valid = loc_nodes < n_loc
        # static per-window graph-selection matrix [128, n_win, G+1]:
        # selg[p, w, j] = 1 if node (w,p) valid and in graph j; col G = valid
        selg = np.zeros((128, n_win, cfg.g + 1), dtype=BFNP)
        bg2 = bglob.reshape(n_win, 128).T
        vd2 = valid.reshape(n_win, 128).T
        for w in range(n_win):
            for p in range(128):
                if vd2[p, w]:
                    selg[p, w, bg2[p, w]] = 1.0
                    selg[p, w, cfg.g] = 1.0

        dsh = slot_shift.reshape(n_chunks_pad, 128).T  # [s, k] float32
        # static transposed selection matrices: sel0T[d, k, s] = (dsh[s,k]==d)
        sel0t = (dsh[:, :, None] == np.arange(128, dtype=np.float32)).astype(BFNP)
        sel0t = np.ascontiguousarray(sel0t.transpose(2, 1, 0))

        per_dev.append(dict(
            src16=wrap16(slot_src - cfg.base),
            dst_shift=dsh.astype(BFNP).copy(),
            sel0t=sel0t,
            selg=selg,
        ))
    return per_dev, n_chunks_pad


def build_program(cfg: Cfg, n_chunks_pad: int, scratch=16384):
    ndev, n_win, cw = cfg.ndev, cfg.n_win, cfg.cw
    dims_in, dims_out = cfg.dims_in, cfg.dims_out
    NL = n_win * 128
    K = n_chunks_pad
    n_layers = len(dims_in)
    Gn = cfg.g
    TW = cfg.tw

    nc = bacc.Bacc("TRN2", target_bir_lowering=False, debug=False,
                   enable_asserts=False, num_devices=ndev,
                   dynamic_dma_scratch_size=scratch, num_swdge_queues=(1 if SIM_MODE else 4))

    def din(name, shape, dt=F32):
        return nc.dram_tensor(name, shape, dt, kind="ExternalInput").ap()

    xt_in = din("xt_in", [128, NL], BF16)
    src16_in = din("src16_in", [128, K * 8], I16)
    sel0t_in = din("sel0t_in", [128, K, 128], BF16)
    dshift_in = din("dshift_in", [128, K], BF16)
    selg_in = din("selg_in", [128, n_win, Gn + 1], BF16)
    cntrep_in = din("cntrep_in", [128, Gn])
    iota8_in = din("iota8_in", [128, SG_CHUNKS * 128], BF16)
    onesrow_in = din("onesrow_in", [1, 128])
    w_in, arep_in, blc_in, g_in, be_in = {}, {}, {}, {}, {}
    for i in range(1, n_layers + 1):
        di, do = dims_in[i - 1], dims_out[i - 1]
        w_in[i] = din(f"wcat{i}_in", [di, 2 * do], BF16 if i == 1 else F32)
        arep_in[i] = din(f"arep{i}_in", [128, TW], BF16)
        blc_in[i] = din(f"blc{i}_in", [1, 2 * do])
        g_in[i] = din(f"g{i}_in", [do, 1])
        be_in[i] = din(f"be{i}_in", [do, 1])
    w5_in = din("w5_in", [224, 128])
    b5_in = din("b5_in", [128, 1])
    g5_in = din("g5_in", [128, 1])
    be5_in = din("be5_in", [128, 1])
    w6_in = din("w6_in", [128, 10])
    b6_in = din("b6_in", [10, 1])

    out_dram = nc.dram_tensor("out", [2, Gn, 10], F32, kind="ExternalOutput").ap()

    nc.gpsimd.load_library(_mlp_lib)

    with tile.TileContext(nc) as tc:
        with tc.tile_pool(name="const", bufs=1) as cst, \
             tc.tile_pool(name="persist", bufs=1) as per, \
             tc.tile_pool(name="dram", bufs=1, space="DRAM") as dram:

            iota8 = cst.tile([128, SG_CHUNKS, 128], BF16)
            nc.sync.dma_start(iota8[:].rearrange("p a b -> p (a b)"), iota8_in[:])
            onesrow = cst.tile([1, 128], F32)
            nc.sync.dma_start(onesrow[:], onesrow_in[:])
            ident = cst.tile([128, 128], F32)
            make_identity(nc, ident[:])
            identb = cst.tile([128, 128], BF16)
            make_identity(nc, identb[:])
            epscol = cst.tile([128, 1], F32)
            nc.vector.memset(epscol[:], BN_EPS)
            xt = per.tile([128, NL], BF16)
            nc.sync.dma_start(xt[:], xt_in[:])
            src16 = per.tile([128, K * 8], I16)
            nc.sync.dma_start(src16[:], src16_in[:])
            dshift = per.tile([128, K], BF16)
            nc.sync.dma_start(dshift[:], dshift_in[:])
            selg_all = per.tile([128, n_win, Gn + 1], BF16)
            nc.sync.dma_start(
                selg_all[:].rearrange("p a b -> p (a b)"),
                selg_in[:].rearrange("p a b -> p (a b)"))

            ybuf = {i: per.tile([128, n_win, dims_out[i - 1]], BF16,
                                name=f"ybuf{i}")
                    for i in range(1, n_layers + 1)}
            xrw = {i: per.tile([128, n_win, dims_out[i - 1]], BF16,
                               name=f"xrw{i}")
                   for i in range(1, n_layers + 1)}
            arep = {}
            for i in range(1, n_layers + 1):
                arep[i] = per.tile([128, TW], BF16, name=f"arep{i}")
                nc.sync.dma_start(arep[i][:], arep_in[i][:])
            pool_sb = {i: per.tile([dims_out[i - 1], Gn + 1], F32, name=f"pool{i}")
                       for i in range(1, n_layers + 1)}
            s2_sb = {i: per.tile([dims_out[i - 1], 1], F32, name=f"s2_{i}")
                     for i in range(1, n_layers + 1)}
            alpha = {i: per.tile([dims_out[i - 1], 1], F32, name=f"alpha{i}")
                     for i in range(1, n_layers + 1)}
            beta = {i: per.tile([dims_out[i - 1], 1], F32, name=f"beta{i}")
                    for i in range(1, n_layers + 1)}

            xl_dram, xltab_dram, st_dram, st_shared = {}, {}, {}, {}
            for i in range(1, n_layers + 1):
                do = dims_out[i - 1]
                xl_dram[i] = dram.tile([NL, TW], BF16, name=f"xld{i}")
                xltab_dram[i] = dram.tile([cfg.n_tab, TW], BF16,
                                          addr_space="Shared", name=f"xltab{i}")
                st_dram[i] = dram.tile([do, 2], F32, name=f"std{i}")
                st_shared[i] = dram.tile([do, 2], F32, addr_space="Shared",
                                         name=f"sts{i}")
            hcat_dram = dram.tile([224, Gn], F32, name="hcatd")
            hcat_shared = dram.tile([224, Gn], F32, addr_space="Shared",
                                    name="hcats")
            rg = [list(range(ndev))]

            sg_counter = [0]
            for li in range(1, n_layers + 1):
                di, do = dims_in[li - 1], dims_out[li - 1]
                _dense_phase(nc, tc, cfg, li, di, do, xt, ybuf, w_in,
                             blc_in, alpha, beta, xl_dram[li], xrw[li],
                             identb, onesrow, n_win)
                nc.gpsimd.collective_compute(
                    "AllGather", OP.bypass, replica_groups=rg,
                    ins=[xl_dram[li][:]], outs=[xltab_dram[li][:]])
                _edge_phase(nc, tc, cfg, li, do, K, n_win, cw,
                            xltab_dram[li], xrw[li], sel0t_in, src16, dshift,
                            arep[li], iota8, ybuf[li], pool_sb[li], s2_sb[li],
                            selg_all, sg_counter)
                nc.sync.dma_start(st_dram[li][:, 0:1], pool_sb[li][:, Gn:Gn + 1])
                nc.sync.dma_start(st_dram[li][:, 1:2], s2_sb[li][:])
                nc.gpsimd.collective_compute(
                    "AllReduce", OP.add, replica_groups=rg,
                    ins=[st_dram[li][:]], outs=[st_shared[li][:]])
                _bn_coeffs(nc, tc, cfg, li, do, st_shared[li], g_in[li],
                           be_in[li], alpha[li], beta[li], epscol)

            _head(nc, tc, cfg, pool_sb, alpha, beta, cntrep_in, hcat_dram,
                  hcat_shared, w5_in, b5_in, g5_in, be5_in, w6_in, b6_in,
                  onesrow, ident, out_dram, rg, epscol)

    nc.compile()
    return nc


def _dense_phase(nc, tc, cfg, li, di, do, xt, ybuf, w_in, blc_in,
                 alpha, beta, xl_d, xrw_l, identb, onesrow, n_win):
    TW = cfg.tw
    with tc.tile_pool(name=f"dn{li}", bufs=3) as sb, \
         tc.tile_pool(name=f"dnp{li}", bufs=2, space="PSUM") as ps, \
         tc.tile_pool(name=f"dnw{li}", bufs=1) as wp:
        wcat = wp.tile([di, 2 * do], BF16)
        bias_rep = wp.tile([128, 2 * do], BF16)
        brow = wp.tile([1, 2 * do], F32)
        blc = wp.tile([1, 2 * do], F32)
        nc.sync.dma_start(blc[:], blc_in[li][:])
        if li == 1:
            nc.sync.dma_start(wcat[:], w_in[1][:])
            nc.vector.tensor_copy(brow[:], blc[:])
        else:
            wraw = wp.tile([di, 2 * do], F32)
            nc.sync.dma_start(wraw[:], w_in[li][:])
            nc.vector.tensor_scalar(out=wcat[:], in0=wraw[:],
                                    scalar1=alpha[li - 1][:], scalar2=None,
                                    op0=OP.mult)
            brow_ps = ps.tile([1, 2 * do], F32, space="PSUM", tag="brow", bufs=1)
            nc.tensor.matmul(brow_ps[:], lhsT=beta[li - 1][:], rhs=wraw[:],
                             start=True, stop=True)
            nc.vector.tensor_tensor(out=brow[:], in0=brow_ps[:], in1=blc[:],
                                    op=OP.add)
        bias_ps = ps.tile([128, 2 * do], F32, space="PSUM", tag="bias", bufs=1)
        nc.tensor.matmul(bias_ps[:], lhsT=onesrow[:], rhs=brow[:],
                         start=True, stop=True)
        nc.vector.tensor_copy(bias_rep[:], bias_ps[:])

        xlb = wp.tile([128, n_win, TW], BF16)
        if TW > do:
            nc.vector.memset(xlb[:, :, do:], 0.0)
        for w in range(n_win):
            if li == 1:
                lhs = xt[:, w * 128:(w + 1) * 128]
            else:
                tr_ps = ps.tile([di, 128], BF16, space="PSUM", tag="tr")
                nc.tensor.transpose(out=tr_ps[:], in_=ybuf[li - 1][:, w, :],
                                    identity=identb[:])
                tr = sb.tile([di, 128], BF16, tag="tr_sb")
                nc.scalar.activation(tr[:], tr_ps[:], AF.Copy)
                lhs = tr[:]
            o_ps = ps.tile([128, 2 * do], F32, space="PSUM", tag="o")
            nc.tensor.matmul(o_ps[:], lhsT=lhs, rhs=wcat[:], start=True,
                             stop=True)
            nc.vector.tensor_tensor(out=xlb[:, w, :do], in0=o_ps[:, :do],
                                    in1=bias_rep[:, :do], op=OP.add)
            nc.vector.tensor_tensor(out=xrw_l[:, w, :], in0=o_ps[:, do:],
                                    in1=bias_rep[:, do:], op=OP.add)
        nc.sync.dma_start(xl_d[:].rearrange("(w p) d -> p w d", p=128), xlb[:])


def _edge_phase(nc, tc, cfg, li, do, K, n_win, cw, xltab, xrw_l, sel0t_in,
                src16, dshift, arep_l, iota8, ybuf_l, pool_l, s2_l,
                selg_all, sg_counter):
    n_chunks = n_win * cw
    Gn = cfg.g
    TW = cfg.tw
    EW = do + 1  # aggregated width: payload + softmax-denominator column
    with tc.tile_pool(name=f"eg{li}", bufs=2) as gb, \
         tc.tile_pool(name=f"et{li}", bufs=2) as tb, \
         tc.tile_pool(name=f"es{li}", bufs=3) as eb, \
         tc.tile_pool(name=f"ea{li}", bufs=2, space="PSUM") as aps, \
         tc.tile_pool(name=f"epp{li}", bufs=1, space="PSUM") as pps:
        pool_ps = pps.tile([do, Gn + 1], F32, space="PSUM", name=f"poolps{li}")
        s2_ps = pps.tile([do, 1], F32, space="PSUM", name=f"s2ps{li}")

        win_psums = {}
        for c0 in range(0, K, SG_CHUNKS):
            c1 = min(c0 + SG_CHUNKS, K)
            nsg = c1 - c0
            ni = nsg * 128
            sgi = sg_counter[0]
            sg_counter[0] += 1
            mbuf = gb.tile([128, nsg, TW], BF16, tag="m")
            nc.gpsimd.dma_gather(mbuf[:], xltab[cfg.base:, :],
                                 src16[:, c0 * 8:c1 * 8], ni, ni, TW,
                                 queue_num=0 if SIM_MODE else sgi % 4)
            selT = gb.tile([128, nsg, 128], BF16, tag="st")
            nc.sync.dma_start(
                selT[:].rearrange("p a b -> p (a b)"),
                sel0t_in[:, c0:c1, :].rearrange("p a b -> p (a b)"))
            # xr[dst] per slot via static selection matmuls, accumulated with
            # the gathered src rows in PSUM
            xr_ps = aps.tile([128, nsg, do], F32, space="PSUM", tag="xrps")
            for c in range(nsg):
                w = (c0 + c) // cw
                nc.tensor.matmul(xr_ps[:, c, :], lhsT=selT[:, c, :],
                                 rhs=xrw_l[:, min(w, n_win - 1), :],
                                 start=True, stop=True)
            # scores: e = a . lrelu(xl[src] + xr[dst])
            tt = eb.tile([128, nsg, do], BF16, tag="tt")
            nc.vector.tensor_tensor(out=tt[:], in0=mbuf[:, :, :do],
                                    in1=xr_ps[:], op=OP.add)
            uu = eb.tile([128, nsg, do], BF16, tag="uu")
            nc.vector.scalar_tensor_tensor(
                out=uu[:], in0=tt[:], scalar=0.2,
                in1=tt[:], op0=OP.mult, op1=OP.max)
            az = eb.tile([128, nsg, do], BF16, tag="az")
            nc.vector.tensor_tensor(
                out=az[:], in0=uu[:],
                in1=arep_l[:, None, :do].to_broadcast([128, nsg, do]),
                op=OP.mult)
            hd = do // 2
            azh = eb.tile([128, nsg, hd], BF16, tag="azh")
            nc.vector.tensor_tensor(out=azh[:], in0=az[:, :, :hd],
                                    in1=az[:, :, hd:do], op=OP.add)
            sg = eb.tile([128, nsg], F32, tag="sg")
            nc.vector.tensor_reduce(out=sg[:], in_=azh[:],
                                    axis=mybir.AxisListType.X, op=OP.add)
            pg = eb.tile([128, nsg], BF16, tag="pg")
            nc.scalar.activation(pg[:], sg[:], AF.Exp)
            # softmax-weighted payload (+ weight itself in column `do`)
            tts = tb.tile([128, nsg, EW], BF16, tag="tts")
            if do < TW:
                nc.vector.memset(mbuf[:, :, do:do + 1], 1.0)
                nc.vector.scalar_tensor_tensor(
                    out=tts[:], in0=mbuf[:, :, :EW], scalar=1.0,
                    in1=pg[:, :, None].to_broadcast([128, nsg, EW]),
                    op0=OP.mult, op1=OP.mult)
            else:
                nc.vector.scalar_tensor_tensor(
                    out=tts[:, :, :do], in0=mbuf[:], scalar=1.0,
                    in1=pg[:, :, None].to_broadcast([128, nsg, do]),
                    op0=OP.mult, op1=OP.mult)
                nc.scalar.activation(tts[:, :, do:do + 1], pg[:, :, None],
                                     AF.Copy)
            sel0 = tb.tile([128, nsg, 128], BF16, tag="sel0")
            nc.vector.scalar_tensor_tensor(
                out=sel0[:], in0=iota8[:, :nsg, :], scalar=1.0,
                in1=dshift[:, c0:c1, None].to_broadcast([128, nsg, 128]),
                op0=OP.mult, op1=OP.is_equal)
            for c in range(nsg):
                k = c0 + c
                if k >= n_chunks:
                    break
                w = k // cw
                if k % cw == 0:
                    win_psums[w] = aps.tile([128, EW], F32, space="PSUM",
                                            tag="agg", name=f"agg{li}_{w}")
                first, last = (k % cw == 0), (k % cw == cw - 1)
                nc.tensor.matmul(win_psums[w][:], lhsT=sel0[:, c, :],
                                 rhs=tts[:, c, :], start=first, stop=last)
                if last:
                    _window_epilogue(nc, eb, cfg, do, w, win_psums.pop(w),
                                     ybuf_l, selg_all, pool_ps, s2_ps,
                                     n_win, Gn)
        nc.vector.tensor_copy(pool_l[:], pool_ps[:])
        nc.vector.tensor_copy(s2_l[:], s2_ps[:])


def _window_epilogue(nc, eb, cfg, do, w, agg_ps, ybuf_l, selg_all,
                     pool_ps, s2_ps, n_win, Gn):
    recip = eb.tile([128, 1], F32, tag="recip")
    nc.vector.reciprocal_approx_fast(recip[:], agg_ps[:, do:do + 1])
    nc.scalar.activation(ybuf_l[:, w, :], agg_ps[:, :do], AF.Relu,
                         scale=recip[:])
    y2 = eb.tile([128, do], BF16, tag="y2")
    nc.scalar.activation(y2[:], ybuf_l[:, w, :], AF.Square)
    nc.tensor.matmul(pool_ps[:], lhsT=ybuf_l[:, w, :], rhs=selg_all[:, w, :],
                     start=(w == 0), stop=(w == n_win - 1))
    nc.tensor.matmul(s2_ps[:], lhsT=y2[:], rhs=selg_all[:, w, Gn:Gn + 1],
                     start=(w == 0), stop=(w == n_win - 1))


def _bn_coeffs(nc, tc, cfg, li, do, st_shared, g_in, be_in, alpha_l, beta_l,
               epscol):
    inv_n = 1.0 / cfg.n
    with tc.tile_pool(name=f"bn{li}", bufs=1) as sb:
        st = sb.tile([do, 2], F32)
        nc.sync.dma_start(st[:], st_shared[:])
        gc = sb.tile([do, 1], F32)
        nc.sync.dma_start(gc[:], g_in[:])
        bec = sb.tile([do, 1], F32)
        nc.sync.dma_start(bec[:], be_in[:])
        mean = sb.tile([do, 1], F32)
        nc.vector.tensor_scalar(out=mean[:], in0=st[:, 0:1], scalar1=inv_n,
                                scalar2=None, op0=OP.mult)
        msq = sb.tile([do, 1], F32)
        nc.vector.tensor_tensor(out=msq[:], in0=mean[:], in1=mean[:], op=OP.mult)
        var = sb.tile([do, 1], F32)
        nc.vector.scalar_tensor_tensor(out=var[:], in0=st[:, 1:2], scalar=inv_n,
                                       in1=msq[:], op0=OP.mult, op1=OP.subtract)
        sd = sb.tile([do, 1], F32)
        nc.scalar.activation(sd[:], var[:], AF.Sqrt, bias=epscol[:do, :])
        rsd = sb.tile([do, 1], F32)
        nc.vector.reciprocal(rsd[:], sd[:])
        nc.vector.tensor_tensor(out=alpha_l[:], in0=gc[:], in1=rsd[:], op=OP.mult)
        ma = sb.tile([do, 1], F32)
        nc.vector.tensor_tensor(out=ma[:], in0=mean[:], in1=alpha_l[:],
                                op=OP.mult)
        nc.vector.tensor_tensor(out=beta_l[:], in0=bec[:], in1=ma[:],
                                op=OP.subtract)


def _head(nc, tc, cfg, pool_sb, alpha, beta, cntrep_in, hcat_dram, hcat_shared,
          w5_in, b5_in, g5_in, be5_in, w6_in, b6_in, onesrow, ident, out_dram,
          rg, epscol):
    Gn = cfg.g
    dims_out = cfg.dims_out
    row_off = [0, 128, 192]
    with tc.tile_pool(name="hd", bufs=1) as sb, \
         tc.tile_pool(name="hdp", bufs=2, space="PSUM") as ps:
        cnt_rep = sb.tile([128, Gn], F32)
        nc.sync.dma_start(cnt_rep[:], cntrep_in[:])
        for i in range(1, len(dims_out) + 1):
            do = dims_out[i - 1]
            pf = sb.tile([do, Gn], F32, name=f"pf{i}")
            nc.vector.tensor_scalar(out=pf[:], in0=pool_sb[i][:, :Gn],
                                    scalar1=alpha[i][:], scalar2=None,
                                    op0=OP.mult)
            nc.vector.scalar_tensor_tensor(out=pf[:], in0=cnt_rep[:do, :],
                                           scalar=beta[i][:], in1=pf[:],
                                           op0=OP.mult, op1=OP.add)
            nc.sync.dma_start(hcat_dram[row_off[i - 1]:row_off[i - 1] + do, :],
                              pf[:])
        nc.gpsimd.collective_compute(
            "AllReduce", OP.add, replica_groups=rg,
            ins=[hcat_dram[:]], outs=[hcat_shared[:]])

        hc_top = sb.tile([128, Gn], F32)
        nc.sync.dma_start(hc_top[:], hcat_shared[0:128, :])
        hc_bot = sb.tile([96, Gn], F32)
        nc.sync.dma_start(hc_bot[:], hcat_shared[128:224, :])
        w5a = sb.tile([128, 128], F32)
        nc.sync.dma_start(w5a[:], w5_in[0:128, :])
        w5b = sb.tile([96, 128], F32)
        nc.sync.dma_start(w5b[:], w5_in[128:224, :])
        b5 = sb.tile([128, 1], F32)
        nc.sync.dma_start(b5[:], b5_in[:])
        h5_ps = ps.tile([128, Gn], F32, space="PSUM", tag="h5")
        nc.tensor.matmul(h5_ps[:], lhsT=w5a[:], rhs=hc_top[:], start=True,
                         stop=False)
        nc.tensor.matmul(h5_ps[:], lhsT=w5b[:], rhs=hc_bot[:], start=False,
                         stop=True)
        h5 = sb.tile([128, Gn], F32)
        nc.scalar.activation(h5[:], h5_ps[:], AF.Relu, bias=b5[:])
        # BN over the graph axis (free): biased var, eps
        scr = sb.tile([128, Gn], F32)
        s1 = sb.tile([128, 1], F32)
        nc.scalar.activation(scr[:], h5[:], AF.Copy, accum_out=s1[:])
        s2 = sb.tile([128, 1], F32)
        nc.scalar.activation(scr[:], h5[:], AF.Square, accum_out=s2[:])
        inv_g = 1.0 / Gn
        mean = sb.tile([128, 1], F32)
        nc.vector.tensor_scalar(out=mean[:], in0=s1[:], scalar1=inv_g,
                                scalar2=None, op0=OP.mult)
        msq = sb.tile([128, 1], F32)
        nc.vector.tensor_tensor(out=msq[:], in0=mean[:], in1=mean[:], op=OP.mult)
        var = sb.tile([128, 1], F32)
        nc.vector.scalar_tensor_tensor(out=var[:], in0=s2[:], scalar=inv_g,
                                       in1=msq[:], op0=OP.mult, op1=OP.subtract)
        sd = sb.tile([128, 1], F32)
        nc.scalar.activation(sd[:], var[:], AF.Sqrt, bias=epscol[:])
        rsd = sb.tile([128, 1], F32)
        nc.vector.reciprocal(rsd[:], sd[:])
        g5 = sb.tile([128, 1], F32)
        nc.sync.dma_start(g5[:], g5_in[:])
        be5 = sb.tile([128, 1], F32)
        nc.sync.dma_start(be5[:], be5_in[:])
        a5 = sb.tile([128, 1], F32)
        nc.vector.tensor_tensor(out=a5[:], in0=g5[:], in1=rsd[:], op=OP.mult)
        ma = sb.tile([128, 1], F32)
        nc.vector.tensor_tensor(out=ma[:], in0=mean[:], in1=a5[:], op=OP.mult)
        b5n = sb.tile([128, 1], F32)
        nc.vector.tensor_tensor(out=b5n[:], in0=be5[:], in1=ma[:], op=OP.subtract)
        h5n = sb.tile([128, Gn], F32)
        nc.scalar.activation(h5n[:], h5[:], AF.Identity, scale=a5[:], bias=b5n[:])

        w6 = sb.tile([128, 10], F32)
        nc.sync.dma_start(w6[:], w6_in[:])
        b6 = sb.tile([10, 1], F32)
        nc.sync.dma_start(b6[:], b6_in[:])
        lg_ps = ps.tile([10, Gn], F32, space="PSUM", tag="lg")
        nc.tensor.matmul(lg_ps[:], lhsT=w6[:], rhs=h5n[:], start=True, stop=True)
        lg = sb.tile([10, Gn], F32)
        nc.scalar.activation(lg[:], lg_ps[:], AF.Identity, bias=b6[:])

        nblk = Gn // 128 if Gn >= 128 else 1
        blk = min(128, Gn)
        lgn = sb.tile([128, nblk, 10], F32)
        for b in range(nblk):
            t_ps = ps.tile([blk, 10], F32, space="PSUM", tag="tr")
            nc.tensor.transpose(out=t_ps[:], in_=lg[:, b * blk:(b + 1) * blk],
                                identity=ident[:10, :10])
            nc.vector.tensor_copy(lgn[:blk, b, :], t_ps[:])
        sig = sb.tile([128, nblk, 10], F32)
        nc.scalar.activation(sig[:blk], lgn[:blk], AF.Sigmoid)
        mx = sb.tile([128, nblk], F32)
        nc.vector.tensor_reduce(out=mx[:blk], in_=lgn[:blk],
                                axis=mybir.AxisListType.X, op=OP.max)
        dd = sb.tile([128, nblk, 10], F32)
        nc.vector.tensor_tensor(out=dd[:blk], in0=lgn[:blk],
                                in1=mx[:blk, :, None].to_broadcast([blk, nblk, 10]),
                                op=OP.subtract)
        ee = sb.tile([128, nblk, 10], F32)
        nc.scalar.activation(ee[:blk], dd[:blk], AF.Exp)
        ssum = sb.tile([128, nblk], F32)
        nc.vector.tensor_reduce(out=ssum[:blk], in_=ee[:blk],
                                axis=mybir.AxisListType.X, op=OP.add)
        lns = sb.tile([128, nblk], F32)
        nc.scalar.activation(lns[:blk], ssum[:blk], AF.Ln)
        lsm = sb.tile([128, nblk, 10], F32)
        nc.vector.tensor_tensor(out=lsm[:blk], in0=dd[:blk],
                                in1=lns[:blk, :, None].to_broadcast([blk, nblk, 10]),
                                op=OP.subtract)
        nc.sync.dma_start(
            out_dram[0, :, :].rearrange("(w p) c -> p w c", p=blk), sig[:blk])
        nc.sync.dma_start(
            out_dram[1, :, :].rearrange("(w p) c -> p w c", p=blk), lsm[:blk])


# ---------------- host-side input packing & runner ----------------

def _fold_weights(inputs, cfg: Cfg):
    """Host-side static folds (small O(d^2) numpy)."""
    f = {}
    for i in range(1, 4):
        do = cfg.dims_out[i - 1]
        wl = np.asarray(inputs[f"Wl{i}"], np.float32)
        wr = np.asarray(inputs[f"Wr{i}"], np.float32)
        wcat = np.concatenate([wl, wr], axis=1)
        f[f"wcat{i}_in"] = wcat.astype(BFNP) if i == 1 else wcat
        a = np.asarray(inputs[f"a{i}"], np.float32)
        arep = np.zeros((128, cfg.tw), np.float32)
        arep[:, :do] = a[None, :]
        f[f"arep{i}_in"] = arep.astype(BFNP)
        bl = np.asarray(inputs[f"bl{i}"], np.float32)
        br = np.asarray(inputs[f"br{i}"], np.float32)
        bc = np.asarray(inputs[f"bc{i}"], np.float32)
        f[f"blc{i}_in"] = np.concatenate([bl + bc, br - bc])[None, :]
        f[f"g{i}_in"] = np.asarray(inputs[f"g{i}"], np.float32)[:, None]
        f[f"be{i}_in"] = np.asarray(inputs[f"be{i}"], np.float32)[:, None]
    w5 = np.asarray(inputs["W5"], np.float32)
    w5eff = w5[:224].copy()
    w5eff[192:224] += w5[224:256]
    f["w5_in"] = w5eff
    f["b5_in"] = np.asarray(inputs["b5"], np.float32)[:, None]
    f["g5_in"] = np.asarray(inputs["g5"], np.float32)[:, None]
    f["be5_in"] = np.asarray(inputs["be5"], np.float32)[:, None]
    f["w6_in"] = np.asarray(inputs["W6"], np.float32)
    f["b6_in"] = np.asarray(inputs["b6"], np.float32)[:, None]
    return f


def build_in_maps(inputs, cfg: Cfg, per_dev):
    x = np.asarray(inputs["x"], np.float32)
    batch = np.asarray(inputs["batch"], np.int64)
    folds = _fold_weights(inputs, cfg)
    cnt = np.bincount(batch, minlength=cfg.g).astype(np.float32)
    cnt_rep = np.tile(cnt[None, :], (128, 1))
    iota8 = np.tile(np.arange(128, dtype=np.float32), (128, SG_CHUNKS)).astype(BFNP)
    onesrow = np.ones((1, 128), np.float32)
    in_maps = []
    zeros_cnt = np.zeros_like(cnt_rep)
    for d in range(cfg.ndev):
        pd = per_dev[d]
        xs = np.zeros((cfg.n_pad, 128), np.float32)
        xs[:cfg.n_loc] = x[d * cfg.n_loc:(d + 1) * cfg.n_loc]
        xt = xs.T.astype(BFNP).copy()
        im = dict(xt_in=xt, src16_in=pd["src16"], sel0t_in=pd["sel0t"],
                  dshift_in=pd["dst_shift"], selg_in=pd["selg"],
                  cntrep_in=cnt_rep if d == 0 else zeros_cnt,
                  iota8_in=iota8,
                  onesrow_in=onesrow, **folds)
        in_maps.append(im)
    return in_maps


_CACHE = {}


def _get_program(cfg: Cfg, n_chunks_pad: int):
    key = (cfg.n, cfg.e, cfg.g, cfg.ndev, cfg.cw, n_chunks_pad)
    if key not in _CACHE:
        _CACHE[key] = build_program(cfg, n_chunks_pad)
    return _CACHE[key]


def _maybe_profile():
    """Optional NTFF capture driven by GAT_PROFILE_DIR (self-contained)."""
    import contextlib
    d = os.environ.get("GAT_PROFILE_DIR")
    if not d:
        return contextlib.nullcontext()
    import ctypes
    import glob

    os.makedirs(d, exist_ok=True)
    for f in glob.glob(d + "/*"):
        os.remove(f)
    lib = ctypes.CDLL("/opt/axon/libaxon_pjrt.so")
    lib.axon_start_nrt_profile.argtypes = [ctypes.POINTER(ctypes.c_int64),
                                           ctypes.c_size_t]
    lib.axon_start_nrt_profile.restype = ctypes.c_int64
    lib.axon_stop_nrt_profile.argtypes = [ctypes.c_char_p]
    lib.axon_stop_nrt_profile.restype = ctypes.c_int64

    @contextlib.contextmanager
    def ctx():
        import jax
        jax.devices()
        rc = lib.axon_start_nrt_profile(None, 0)
        if rc != 0:
            raise RuntimeError(f"profile start rc={rc}")
        try:
            yield
        finally:
            lib.axon_stop_nrt_profile(str(d).encode())

    return ctx()


def kernel(**inputs):
    cfg = Cfg()
    per_dev, n_chunks_pad = preprocess(inputs["edge_index"], inputs["batch"], cfg)
    nc = _get_program(cfg, n_chunks_pad)
    in_maps = build_in_maps(inputs, cfg, per_dev)
    with _maybe_profile():
        res = bass_utils.run_bass_kernel_spmd(nc, in_maps,
                                              core_ids=list(range(cfg.ndev)))
    out = np.asarray(res.results[0]["out"])
    return (out[0], out[1])


# revision 18
# speedup vs baseline: 2.8411x; 1.6372x over previous
"""GATv2 (3 live layers) + sum-pooling + MLP head on 8 Trainium2 NeuronCores.

Sharding: nodes/edges sharded by destination-node range across 8 cores.
Per layer (all tables bf16): sharded dense phase (xl/xr projections, BN folded
into weights), AllGather of the xl table, edge phase with two large batched
dma_gathers per 64-chunk supergroup (src rows from the global table, dst rows
from the local table), scores via scalar-engine LeakyReLU + vector mult/reduce,
softmax weights folded into the gathered payload, selection-matrix bf16 matmul
aggregation into aligned 128-node windows (denominator via a ones column),
PE-accumulated global pools/stats, fp32 MLP head replicated on every core.

Self-contained: hardcodes shapes from the problem spec; host side does only
integer index preprocessing and O(d^2) weight folds.
"""
import os
import sys

sys.path.insert(0, "/opt/trn_rl_repo")

import ml_dtypes
import numpy as np

import concourse.bass as bass
import concourse.bacc as bacc
import concourse.tile as tile
from concourse import mybir
from concourse import bass_utils
from concourse.library_config import mlp as _mlp_lib
from concourse.masks import make_identity

F32 = mybir.dt.float32
BF16 = mybir.dt.float16
I16 = mybir.dt.int16
OP = mybir.AluOpType
AF = mybir.ActivationFunctionType
BFNP = np.float16

N, E, GRAPHS = 50000, 400000, 256
NDEV = 8
MID = 32768
PAD_SHIFT = 60000.0
BN_EPS = 1e-5
SG_CHUNKS = 8  # chunks per gather supergroup (8*128 = 1024 indices)
SIM_MODE = False  # simtest sets True: single SWDGE queue to satisfy CoreSim's lane check


class Cfg:
    def __init__(self, n=N, e=E, g=GRAPHS, ndev=NDEV):
        assert n % ndev == 0
        self.n, self.e, self.g, self.ndev = n, e, g, ndev
        self.n_loc = n // ndev
        self.n_win = (self.n_loc + 127) // 128
        self.n_pad = self.n_win * 128
        self.n_tab = self.n_pad * ndev
        self.cw = None
        self.base = MID if self.n_tab > 32000 else 0
        self.iw = max(g, 128)
        self.dims_in = [128, 128, 64]
        self.dims_out = [128, 64, 32]
        self.tw = 128  # table width (bf16 rows must be 256B-aligned)


def preprocess(edge_index, batch, cfg: Cfg):
    n, ndev, n_loc, n_win = cfg.n, cfg.ndev, cfg.n_loc, cfg.n_win
    src = np.concatenate([np.asarray(edge_index[0]), np.arange(n)]).astype(np.int64)
    dst = np.concatenate([np.asarray(edge_index[1]), np.arange(n)]).astype(np.int64)
    batch = np.asarray(batch).astype(np.int64)

    def pad_id(x):
        return (x // n_loc) * (n_win * 128) + (x % n_loc)

    dev_of = dst // n_loc
    dev_data = []
    max_cw = 1
    for d in range(ndev):
        m = dev_of == d
        s_d, t_d = src[m], dst[m] - d * n_loc
        padn = np.arange(n_loc, n_win * 128)
        s_d = np.concatenate([s_d, np.full(len(padn), d * n_loc)])
        t_d = np.concatenate([t_d, padn])
        order = np.argsort(t_d, kind="stable")
        s_d, t_d = s_d[order], t_d[order]
        cnts = np.bincount(t_d // 128, minlength=n_win)
        max_cw = max(max_cw, int(np.ceil(cnts.max() / 128)))
        dev_data.append((s_d, t_d, cnts))

    cfg.cw = cw = max_cw
    n_chunks = n_win * cw
    n_chunks_pad = ((n_chunks + 7) // 8) * 8
    L = n_chunks_pad * 128

    per_dev = []
    for d in range(ndev):
        s_d, t_d, cnts = dev_data[d]
        slot_src = np.full(L, cfg.base, dtype=np.int64)
        slot_rdst = np.zeros(L, dtype=np.int64)
        slot_shift = np.full(L, PAD_SHIFT, dtype=np.float32)
        pos = 0
        for w in range(n_win):
            cnt = int(cnts[w])
            base = w * cw * 128
            sl = slice(pos, pos + cnt)
            slot_src[base:base + cnt] = pad_id(s_d[sl])
            slot_rdst[base:base + cnt] = t_d[sl]
            slot_shift[base:base + cnt] = (t_d[sl] - w * 128).astype(np.float32)
            pos += cnt
        assert pos == len(s_d)

        # each 1024-slot gather block must end with a non-negative (src-base)
        # index: trailing negative int16 idxs are dropped by the gather ucode.
        if cfg.base > 0:
            for gb in range(0, L, 1024):
                if slot_src[gb + 1023] - cfg.base < 0:
                    cand = np.where(slot_src[gb:gb + 1024] - cfg.base >= 0)[0]
                    assert len(cand), "gather group has no non-negative index"
                    j = gb + cand[-1]
                    for arr in (slot_src, slot_rdst, slot_shift):
                        arr[j], arr[gb + 1023] = arr[gb + 1023], arr[j]

        def wrap16(vals):
            v = vals.astype(np.int16)
            return np.tile(v.reshape(-1, 16).T, (8, 1)).copy()

        loc_nodes = np.arange(n_win * 128)
        glob_nodes = np.minimum(d * n_loc + loc_nodes, n - 1)
        bglob = batch[glob_nodes]
        valid = loc_nodes < n_loc
        # static per-window graph-selection matrix [128, n_win, G+1]:
        # selg[p, w, j] = 1 if node (w,p) valid and in graph j; col G = valid
        selg = np.zeros((128, n_win, cfg.g + 1), dtype=BFNP)
        bg2 = bglob.reshape(n_win, 128).T
        vd2 = valid.reshape(n_win, 128).T
        for w in range(n_win):
            for p in range(128):
                if vd2[p, w]:
                    selg[p, w, bg2[p, w]] = 1.0
                    selg[p, w, cfg.g] = 1.0

        dsh = slot_shift.reshape(n_chunks_pad, 128).T  # [s, k] float32
        # static transposed selection matrices: sel0T[d, k, s] = (dsh[s,k]==d)
        sel0t = (dsh[:, :, None] == np.arange(128, dtype=np.float32)).astype(BFNP)
        sel0t = np.ascontiguousarray(sel0t.transpose(2, 1, 0))

        per_dev.append(dict(
            src16=wrap16(slot_src - cfg.base),
            dst_shift=dsh.astype(BFNP).copy(),
            sel0t=sel0t,
            selg=selg,
        ))
    return per_dev, n_chunks_pad


def build_program(cfg: Cfg, n_chunks_pad: int, scratch=16384):
    ndev, n_win, cw = cfg.ndev, cfg.n_win, cfg.cw
    dims_in, dims_out = cfg.dims_in, cfg.dims_out
    NL = n_win * 128
    K = n_chunks_pad
    n_layers = len(dims_in)
    Gn = cfg.g
    TW = cfg.tw

    nc = bacc.Bacc("TRN2", target_bir_lowering=False, debug=False,
                   enable_asserts=False, num_devices=ndev,
                   dynamic_dma_scratch_size=scratch, num_swdge_queues=(1 if SIM_MODE else 4))

    def din(name, shape, dt=F32):
        return nc.dram_tensor(name, shape, dt, kind="ExternalInput").ap()

    xt_in = din("xt_in", [128, NL], BF16)
    src16_in = din("src16_in", [128, K * 8], I16)
    sel0t_in = din("sel0t_in", [128, K, 128], BF16)
    dshift_in = din("dshift_in", [128, K], BF16)
    selg_in = din("selg_in", [128, n_win, Gn + 1], BF16)
    cntrep_in = din("cntrep_in", [128, Gn])
    iota8_in = din("iota8_in", [128, SG_CHUNKS * 128], BF16)
    onesrow_in = din("onesrow_in", [1, 128])
    w_in, arep_in, blc_in, g_in, be_in = {}, {}, {}, {}, {}
    for i in range(1, n_layers + 1):
        di, do = dims_in[i - 1], dims_out[i - 1]
        w_in[i] = din(f"wcat{i}_in", [di, 2 * do], BF16 if i == 1 else F32)
        arep_in[i] = din(f"arep{i}_in", [128, TW], BF16)
        blc_in[i] = din(f"blc{i}_in", [1, 2 * do])
        g_in[i] = din(f"g{i}_in", [do, 1])
        be_in[i] = din(f"be{i}_in", [do, 1])
    w5_in = din("w5_in", [224, 128])
    b5_in = din("b5_in", [128, 1])
    g5_in = din("g5_in", [128, 1])
    be5_in = din("be5_in", [128, 1])
    w6_in = din("w6_in", [128, 10])
    b6_in = din("b6_in", [10, 1])

    out_dram = nc.dram_tensor("out", [2, Gn, 10], F32, kind="ExternalOutput").ap()

    nc.gpsimd.load_library(_mlp_lib)

    with tile.TileContext(nc) as tc:
        with tc.tile_pool(name="const", bufs=1) as cst, \
             tc.tile_pool(name="persist", bufs=1) as per, \
             tc.tile_pool(name="dram", bufs=1, space="DRAM") as dram:

            iota8 = cst.tile([128, SG_CHUNKS, 128], BF16)
            nc.sync.dma_start(iota8[:].rearrange("p a b -> p (a b)"), iota8_in[:])
            onesrow = cst.tile([1, 128], F32)
            nc.sync.dma_start(onesrow[:], onesrow_in[:])
            ident = cst.tile([128, 128], F32)
            make_identity(nc, ident[:])
            identb = cst.tile([128, 128], BF16)
            make_identity(nc, identb[:])
            epscol = cst.tile([128, 1], F32)
            nc.vector.memset(epscol[:], BN_EPS)
            xt = per.tile([128, NL], BF16)
            nc.sync.dma_start(xt[:], xt_in[:])
            src16 = per.tile([128, K * 8], I16)
            nc.sync.dma_start(src16[:], src16_in[:])
            dshift = per.tile([128, K], BF16)
            nc.sync.dma_start(dshift[:], dshift_in[:])
            selg_all = per.tile([128, n_win, Gn + 1], BF16)
            nc.sync.dma_start(
                selg_all[:].rearrange("p a b -> p (a b)"),
                selg_in[:].rearrange("p a b -> p (a b)"))

            ybuf = {i: per.tile([128, n_win, dims_out[i - 1]], BF16,
                                name=f"ybuf{i}")
                    for i in range(1, n_layers + 1)}
            xrw = {i: per.tile([128, n_win, dims_out[i - 1]], BF16,
                               name=f"xrw{i}")
                   for i in range(1, n_layers + 1)}
            arep = {}
            for i in range(1, n_layers + 1):
                arep[i] = per.tile([128, TW], BF16, name=f"arep{i}")
                nc.sync.dma_start(arep[i][:], arep_in[i][:])
            pool_sb = {i: per.tile([dims_out[i - 1], Gn + 1], F32, name=f"pool{i}")
                       for i in range(1, n_layers + 1)}
            s2_sb = {i: per.tile([dims_out[i - 1], 1], F32, name=f"s2_{i}")
                     for i in range(1, n_layers + 1)}
            alpha = {i: per.tile([dims_out[i - 1], 1], F32, name=f"alpha{i}")
                     for i in range(1, n_layers + 1)}
            beta = {i: per.tile([dims_out[i - 1], 1], F32, name=f"beta{i}")
                    for i in range(1, n_layers + 1)}

            xl_dram, xltab_dram, st_dram, st_shared = {}, {}, {}, {}
            for i in range(1, n_layers + 1):
                do = dims_out[i - 1]
                xl_dram[i] = dram.tile([NL, TW], BF16, name=f"xld{i}")
                xltab_dram[i] = dram.tile([cfg.n_tab, TW], BF16,
                                          addr_space="Shared", name=f"xltab{i}")
                st_dram[i] = dram.tile([do, 2], F32, name=f"std{i}")
                st_shared[i] = dram.tile([do, 2], F32, addr_space="Shared",
                                         name=f"sts{i}")
            hcat_dram = dram.tile([224, Gn], F32, name="hcatd")
            hcat_shared = dram.tile([224, Gn], F32, addr_space="Shared",
                                    name="hcats")
            rg = [list(range(ndev))]

            sg_counter = [0]
            for li in range(1, n_layers + 1):
                di, do = dims_in[li - 1], dims_out[li - 1]
                _dense_phase(nc, tc, cfg, li, di, do, xt, ybuf, w_in,
                             blc_in, alpha, beta, xl_dram[li], xrw[li],
                             identb, onesrow, n_win)
                nc.gpsimd.collective_compute(
                    "AllGather", OP.bypass, replica_groups=rg,
                    ins=[xl_dram[li][:]], outs=[xltab_dram[li][:]])
                _edge_phase(nc, tc, cfg, li, do, K, n_win, cw,
                            xltab_dram[li], xrw[li], sel0t_in, src16, dshift,
                            arep[li], iota8, ybuf[li], pool_sb[li], s2_sb[li],
                            selg_all, sg_counter)
                nc.sync.dma_start(st_dram[li][:, 0:1], pool_sb[li][:, Gn:Gn + 1])
                nc.sync.dma_start(st_dram[li][:, 1:2], s2_sb[li][:])
                nc.gpsimd.collective_compute(
                    "AllReduce", OP.add, replica_groups=rg,
                    ins=[st_dram[li][:]], outs=[st_shared[li][:]])
                _bn_coeffs(nc, tc, cfg, li, do, st_shared[li], g_in[li],
                           be_in[li], alpha[li], beta[li], epscol)

            _head(nc, tc, cfg, pool_sb, alpha, beta, cntrep_in, hcat_dram,
                  hcat_shared, w5_in, b5_in, g5_in, be5_in, w6_in, b6_in,
                  onesrow, ident, out_dram, rg, epscol)

    nc.compile()
    return nc


def _dense_phase(nc, tc, cfg, li, di, do, xt, ybuf, w_in, blc_in,
                 alpha, beta, xl_d, xrw_l, identb, onesrow, n_win):
    TW = cfg.tw
    with tc.tile_pool(name=f"dn{li}", bufs=3) as sb, \
         tc.tile_pool(name=f"dnp{li}", bufs=2, space="PSUM") as ps, \
         tc.tile_pool(name=f"dnw{li}", bufs=1) as wp:
        wcat = wp.tile([di, 2 * do], BF16)
        bias_rep = wp.tile([128, 2 * do], BF16)
        brow = wp.tile([1, 2 * do], F32)
        blc = wp.tile([1, 2 * do], F32)
        nc.sync.dma_start(blc[:], blc_in[li][:])
        if li == 1:
            nc.sync.dma_start(wcat[:], w_in[1][:])
            nc.vector.tensor_copy(brow[:], blc[:])
        else:
            wraw = wp.tile([di, 2 * do], F32)
            nc.sync.dma_start(wraw[:], w_in[li][:])
            nc.vector.tensor_scalar(out=wcat[:], in0=wraw[:],
                                    scalar1=alpha[li - 1][:], scalar2=None,
                                    op0=OP.mult)
            brow_ps = ps.tile([1, 2 * do], F32, space="PSUM", tag="brow", bufs=1)
            nc.tensor.matmul(brow_ps[:], lhsT=beta[li - 1][:], rhs=wraw[:],
                             start=True, stop=True)
            nc.vector.tensor_tensor(out=brow[:], in0=brow_ps[:], in1=blc[:],
                                    op=OP.add)
        bias_ps = ps.tile([128, 2 * do], F32, space="PSUM", tag="bias", bufs=1)
        nc.tensor.matmul(bias_ps[:], lhsT=onesrow[:], rhs=brow[:],
                         start=True, stop=True)
        nc.vector.tensor_copy(bias_rep[:], bias_ps[:])

        xlb = wp.tile([128, n_win, TW], BF16)
        if TW > do:
            nc.vector.memset(xlb[:, :, do:], 0.0)
        for w in range(n_win):
            if li == 1:
                lhs = xt[:, w * 128:(w + 1) * 128]
            else:
                tr_ps = ps.tile([di, 128], BF16, space="PSUM", tag="tr")
                nc.tensor.transpose(out=tr_ps[:], in_=ybuf[li - 1][:, w, :],
                                    identity=identb[:])
                tr = sb.tile([di, 128], BF16, tag="tr_sb")
                nc.scalar.activation(tr[:], tr_ps[:], AF.Copy)
                lhs = tr[:]
            o_ps = ps.tile([128, 2 * do], F32, space="PSUM", tag="o")
            nc.tensor.matmul(o_ps[:], lhsT=lhs, rhs=wcat[:], start=True,
                             stop=True)
            nc.vector.tensor_tensor(out=xlb[:, w, :do], in0=o_ps[:, :do],
                                    in1=bias_rep[:, :do], op=OP.add)
            nc.vector.tensor_tensor(out=xrw_l[:, w, :], in0=o_ps[:, do:],
                                    in1=bias_rep[:, do:], op=OP.add)
        nc.sync.dma_start(xl_d[:].rearrange("(w p) d -> p w d", p=128), xlb[:])


def _edge_phase(nc, tc, cfg, li, do, K, n_win, cw, xltab, xrw_l, sel0t_in,
                src16, dshift, arep_l, iota8, ybuf_l, pool_l, s2_l,
                selg_all, sg_counter):
    n_chunks = n_win * cw
    Gn = cfg.g
    TW = cfg.tw
    EW = do + 1  # aggregated width: payload + softmax-denominator column
    with tc.tile_pool(name=f"eg{li}", bufs=4) as gb, \
         tc.tile_pool(name=f"et{li}", bufs=3) as tb, \
         tc.tile_pool(name=f"es{li}", bufs=4) as eb, \
         tc.tile_pool(name=f"ea{li}", bufs=2, space="PSUM") as aps, \
         tc.tile_pool(name=f"epp{li}", bufs=1, space="PSUM") as pps:
        pool_ps = pps.tile([do, Gn + 1], F32, space="PSUM", name=f"poolps{li}")
        s2_ps = pps.tile([do, 1], F32, space="PSUM", name=f"s2ps{li}")

        win_psums = {}
        for c0 in range(0, K, SG_CHUNKS):
            c1 = min(c0 + SG_CHUNKS, K)
            nsg = c1 - c0
            ni = nsg * 128
            sgi = sg_counter[0]
            sg_counter[0] += 1
            mbuf = gb.tile([128, nsg, TW], BF16, tag="m")
            nc.gpsimd.dma_gather(mbuf[:], xltab[cfg.base:, :],
                                 src16[:, c0 * 8:c1 * 8], ni, ni, TW,
                                 queue_num=0 if SIM_MODE else sgi % 4)
            selT = gb.tile([128, nsg, 128], BF16, tag="st")
            nc.sync.dma_start(
                selT[:].rearrange("p a b -> p (a b)"),
                sel0t_in[:, c0:c1, :].rearrange("p a b -> p (a b)"))
            # xr[dst] per slot via static selection matmuls, accumulated with
            # the gathered src rows in PSUM
            xr_ps = aps.tile([128, nsg, do], F32, space="PSUM", tag="xrps")
            for c in range(nsg):
                w = (c0 + c) // cw
                nc.tensor.matmul(xr_ps[:, c, :], lhsT=selT[:, c, :],
                                 rhs=xrw_l[:, min(w, n_win - 1), :],
                                 start=True, stop=True)
            # scores: e = a . lrelu(xl[src] + xr[dst])
            tt = eb.tile([128, nsg, do], BF16, tag="tt")
            nc.vector.tensor_tensor(out=tt[:], in0=mbuf[:, :, :do],
                                    in1=xr_ps[:], op=OP.add)
            uu = eb.tile([128, nsg, do], BF16, tag="uu")
            nc.vector.scalar_tensor_tensor(
                out=uu[:], in0=tt[:], scalar=0.2,
                in1=tt[:], op0=OP.mult, op1=OP.max)
            az = eb.tile([128, nsg, do], BF16, tag="az")
            nc.vector.tensor_tensor(
                out=az[:], in0=uu[:],
                in1=arep_l[:, None, :do].to_broadcast([128, nsg, do]),
                op=OP.mult)
            hd = do // 2
            azh = eb.tile([128, nsg, hd], BF16, tag="azh")
            nc.vector.tensor_tensor(out=azh[:], in0=az[:, :, :hd],
                                    in1=az[:, :, hd:do], op=OP.add)
            sg = eb.tile([128, nsg], F32, tag="sg")
            nc.vector.tensor_reduce(out=sg[:], in_=azh[:],
                                    axis=mybir.AxisListType.X, op=OP.add)
            pg = eb.tile([128, nsg], BF16, tag="pg")
            nc.scalar.activation(pg[:], sg[:], AF.Exp)
            # softmax-weighted payload (+ weight itself in column `do`)
            tts = tb.tile([128, nsg, EW], BF16, tag="tts")
            if do < TW:
                nc.vector.memset(mbuf[:, :, do:do + 1], 1.0)
                nc.vector.scalar_tensor_tensor(
                    out=tts[:], in0=mbuf[:, :, :EW], scalar=1.0,
                    in1=pg[:, :, None].to_broadcast([128, nsg, EW]),
                    op0=OP.mult, op1=OP.mult)
            else:
                nc.vector.scalar_tensor_tensor(
                    out=tts[:, :, :do], in0=mbuf[:], scalar=1.0,
                    in1=pg[:, :, None].to_broadcast([128, nsg, do]),
                    op0=OP.mult, op1=OP.mult)
                nc.scalar.activation(tts[:, :, do:do + 1], pg[:, :, None],
                                     AF.Copy)
            sel0 = tb.tile([128, nsg, 128], BF16, tag="sel0")
            nc.vector.scalar_tensor_tensor(
                out=sel0[:], in0=iota8[:, :nsg, :], scalar=1.0,
                in1=dshift[:, c0:c1, None].to_broadcast([128, nsg, 128]),
                op0=OP.mult, op1=OP.is_equal)
            for c in range(nsg):
                k = c0 + c
                if k >= n_chunks:
                    break
                w = k // cw
                if k % cw == 0:
                    win_psums[w] = aps.tile([128, EW], F32, space="PSUM",
                                            tag="agg", name=f"agg{li}_{w}")
                first, last = (k % cw == 0), (k % cw == cw - 1)
                nc.tensor.matmul(win_psums[w][:], lhsT=sel0[:, c, :],
                                 rhs=tts[:, c, :], start=first, stop=last)
                if last:
                    _window_epilogue(nc, eb, cfg, do, w, win_psums.pop(w),
                                     ybuf_l, selg_all, pool_ps, s2_ps,
                                     n_win, Gn)
        nc.vector.tensor_copy(pool_l[:], pool_ps[:])
        nc.vector.tensor_copy(s2_l[:], s2_ps[:])


def _window_epilogue(nc, eb, cfg, do, w, agg_ps, ybuf_l, selg_all,
                     pool_ps, s2_ps, n_win, Gn):
    recip = eb.tile([128, 1], F32, tag="recip")
    nc.vector.reciprocal_approx_fast(recip[:], agg_ps[:, do:do + 1])
    nc.scalar.activation(ybuf_l[:, w, :], agg_ps[:, :do], AF.Relu,
                         scale=recip[:])
    y2 = eb.tile([128, do], BF16, tag="y2")
    nc.scalar.activation(y2[:], ybuf_l[:, w, :], AF.Square)
    nc.tensor.matmul(pool_ps[:], lhsT=ybuf_l[:, w, :], rhs=selg_all[:, w, :],
                     start=(w == 0), stop=(w == n_win - 1))
    nc.tensor.matmul(s2_ps[:], lhsT=y2[:], rhs=selg_all[:, w, Gn:Gn + 1],
                     start=(w == 0), stop=(w == n_win - 1))


def _bn_coeffs(nc, tc, cfg, li, do, st_shared, g_in, be_in, alpha_l, beta_l,
               epscol):
    inv_n = 1.0 / cfg.n
    with tc.tile_pool(name=f"bn{li}", bufs=1) as sb:
        st = sb.tile([do, 2], F32)
        nc.sync.dma_start(st[:], st_shared[:])
        gc = sb.tile([do, 1], F32)
        nc.sync.dma_start(gc[:], g_in[:])
        bec = sb.tile([do, 1], F32)
        nc.sync.dma_start(bec[:], be_in[:])
        mean = sb.tile([do, 1], F32)
        nc.vector.tensor_scalar(out=mean[:], in0=st[:, 0:1], scalar1=inv_n,
                                scalar2=None, op0=OP.mult)
        msq = sb.tile([do, 1], F32)
        nc.vector.tensor_tensor(out=msq[:], in0=mean[:], in1=mean[:], op=OP.mult)
        var = sb.tile([do, 1], F32)
        nc.vector.scalar_tensor_tensor(out=var[:], in0=st[:, 1:2], scalar=inv_n,
                                       in1=msq[:], op0=OP.mult, op1=OP.subtract)
        sd = sb.tile([do, 1], F32)
        nc.scalar.activation(sd[:], var[:], AF.Sqrt, bias=epscol[:do, :])
        rsd = sb.tile([do, 1], F32)
        nc.vector.reciprocal(rsd[:], sd[:])
        nc.vector.tensor_tensor(out=alpha_l[:], in0=gc[:], in1=rsd[:], op=OP.mult)
        ma = sb.tile([do, 1], F32)
        nc.vector.tensor_tensor(out=ma[:], in0=mean[:], in1=alpha_l[:],
                                op=OP.mult)
        nc.vector.tensor_tensor(out=beta_l[:], in0=bec[:], in1=ma[:],
                                op=OP.subtract)


def _head(nc, tc, cfg, pool_sb, alpha, beta, cntrep_in, hcat_dram, hcat_shared,
          w5_in, b5_in, g5_in, be5_in, w6_in, b6_in, onesrow, ident, out_dram,
          rg, epscol):
    Gn = cfg.g
    dims_out = cfg.dims_out
    row_off = [0, 128, 192]
    with tc.tile_pool(name="hd", bufs=1) as sb, \
         tc.tile_pool(name="hdp", bufs=2, space="PSUM") as ps:
        cnt_rep = sb.tile([128, Gn], F32)
        nc.sync.dma_start(cnt_rep[:], cntrep_in[:])
        for i in range(1, len(dims_out) + 1):
            do = dims_out[i - 1]
            pf = sb.tile([do, Gn], F32, name=f"pf{i}")
            nc.vector.tensor_scalar(out=pf[:], in0=pool_sb[i][:, :Gn],
                                    scalar1=alpha[i][:], scalar2=None,
                                    op0=OP.mult)
            nc.vector.scalar_tensor_tensor(out=pf[:], in0=cnt_rep[:do, :],
                                           scalar=beta[i][:], in1=pf[:],
                                           op0=OP.mult, op1=OP.add)
            nc.sync.dma_start(hcat_dram[row_off[i - 1]:row_off[i - 1] + do, :],
                              pf[:])
        nc.gpsimd.collective_compute(
            "AllReduce", OP.add, replica_groups=rg,
            ins=[hcat_dram[:]], outs=[hcat_shared[:]])

        hc_top = sb.tile([128, Gn], F32)
        nc.sync.dma_start(hc_top[:], hcat_shared[0:128, :])
        hc_bot = sb.tile([96, Gn], F32)
        nc.sync.dma_start(hc_bot[:], hcat_shared[128:224, :])
        w5a = sb.tile([128, 128], F32)
        nc.sync.dma_start(w5a[:], w5_in[0:128, :])
        w5b = sb.tile([96, 128], F32)
        nc.sync.dma_start(w5b[:], w5_in[128:224, :])
        b5 = sb.tile([128, 1], F32)
        nc.sync.dma_start(b5[:], b5_in[:])
        h5_ps = ps.tile([128, Gn], F32, space="PSUM", tag="h5")
        nc.tensor.matmul(h5_ps[:], lhsT=w5a[:], rhs=hc_top[:], start=True,
                         stop=False)
        nc.tensor.matmul(h5_ps[:], lhsT=w5b[:], rhs=hc_bot[:], start=False,
                         stop=True)
        h5 = sb.tile([128, Gn], F32)
        nc.scalar.activation(h5[:], h5_ps[:], AF.Relu, bias=b5[:])
        # BN over the graph axis (free): biased var, eps
        scr = sb.tile([128, Gn], F32)
        s1 = sb.tile([128, 1], F32)
        nc.scalar.activation(scr[:], h5[:], AF.Copy, accum_out=s1[:])
        s2 = sb.tile([128, 1], F32)
        nc.scalar.activation(scr[:], h5[:], AF.Square, accum_out=s2[:])
        inv_g = 1.0 / Gn
        mean = sb.tile([128, 1], F32)
        nc.vector.tensor_scalar(out=mean[:], in0=s1[:], scalar1=inv_g,
                                scalar2=None, op0=OP.mult)
        msq = sb.tile([128, 1], F32)
        nc.vector.tensor_tensor(out=msq[:], in0=mean[:], in1=mean[:], op=OP.mult)
        var = sb.tile([128, 1], F32)
        nc.vector.scalar_tensor_tensor(out=var[:], in0=s2[:], scalar=inv_g,
                                       in1=msq[:], op0=OP.mult, op1=OP.subtract)
        sd = sb.tile([128, 1], F32)
        nc.scalar.activation(sd[:], var[:], AF.Sqrt, bias=epscol[:])
        rsd = sb.tile([128, 1], F32)
        nc.vector.reciprocal(rsd[:], sd[:])
        g5 = sb.tile([128, 1], F32)
        nc.sync.dma_start(g5[:], g5_in[:])
        be5 = sb.tile([128, 1], F32)
        nc.sync.dma_start(be5[:], be5_in[:])
        a5 = sb.tile([128, 1], F32)
        nc.vector.tensor_tensor(out=a5[:], in0=g5[:], in1=rsd[:], op=OP.mult)
        ma = sb.tile([128, 1], F32)
        nc.vector.tensor_tensor(out=ma[:], in0=mean[:], in1=a5[:], op=OP.mult)
        b5n = sb.tile([128, 1], F32)
        nc.vector.tensor_tensor(out=b5n[:], in0=be5[:], in1=ma[:], op=OP.subtract)
        h5n = sb.tile([128, Gn], F32)
        nc.scalar.activation(h5n[:], h5[:], AF.Identity, scale=a5[:], bias=b5n[:])

        w6 = sb.tile([128, 10], F32)
        nc.sync.dma_start(w6[:], w6_in[:])
        b6 = sb.tile([10, 1], F32)
        nc.sync.dma_start(b6[:], b6_in[:])
        lg_ps = ps.tile([10, Gn], F32, space="PSUM", tag="lg")
        nc.tensor.matmul(lg_ps[:], lhsT=w6[:], rhs=h5n[:], start=True, stop=True)
        lg = sb.tile([10, Gn], F32)
        nc.scalar.activation(lg[:], lg_ps[:], AF.Identity, bias=b6[:])

        nblk = Gn // 128 if Gn >= 128 else 1
        blk = min(128, Gn)
        lgn = sb.tile([128, nblk, 10], F32)
        for b in range(nblk):
            t_ps = ps.tile([blk, 10], F32, space="PSUM", tag="tr")
            nc.tensor.transpose(out=t_ps[:], in_=lg[:, b * blk:(b + 1) * blk],
                                identity=ident[:10, :10])
            nc.vector.tensor_copy(lgn[:blk, b, :], t_ps[:])
        sig = sb.tile([128, nblk, 10], F32)
        nc.scalar.activation(sig[:blk], lgn[:blk], AF.Sigmoid)
        mx = sb.tile([128, nblk], F32)
        nc.vector.tensor_reduce(out=mx[:blk], in_=lgn[:blk],
                                axis=mybir.AxisListType.X, op=OP.max)
        dd = sb.tile([128, nblk, 10], F32)
        nc.vector.tensor_tensor(out=dd[:blk], in0=lgn[:blk],
                                in1=mx[:blk, :, None].to_broadcast([blk, nblk, 10]),
                                op=OP.subtract)
        ee = sb.tile([128, nblk, 10], F32)
        nc.scalar.activation(ee[:blk], dd[:blk], AF.Exp)
        ssum = sb.tile([128, nblk], F32)
        nc.vector.tensor_reduce(out=ssum[:blk], in_=ee[:blk],
                                axis=mybir.AxisListType.X, op=OP.add)
        lns = sb.tile([128, nblk], F32)
        nc.scalar.activation(lns[:blk], ssum[:blk], AF.Ln)
        lsm = sb.tile([128, nblk, 10], F32)
        nc.vector.tensor_tensor(out=lsm[:blk], in0=dd[:blk],
                                in1=lns[:blk, :, None].to_broadcast([blk, nblk, 10]),
                                op=OP.subtract)
        nc.sync.dma_start(
            out_dram[0, :, :].rearrange("(w p) c -> p w c", p=blk), sig[:blk])
        nc.sync.dma_start(
            out_dram[1, :, :].rearrange("(w p) c -> p w c", p=blk), lsm[:blk])


# ---------------- host-side input packing & runner ----------------

def _fold_weights(inputs, cfg: Cfg):
    """Host-side static folds (small O(d^2) numpy)."""
    f = {}
    for i in range(1, 4):
        do = cfg.dims_out[i - 1]
        wl = np.asarray(inputs[f"Wl{i}"], np.float32)
        wr = np.asarray(inputs[f"Wr{i}"], np.float32)
        wcat = np.concatenate([wl, wr], axis=1)
        f[f"wcat{i}_in"] = wcat.astype(BFNP) if i == 1 else wcat
        a = np.asarray(inputs[f"a{i}"], np.float32)
        arep = np.zeros((128, cfg.tw), np.float32)
        arep[:, :do] = a[None, :]
        f[f"arep{i}_in"] = arep.astype(BFNP)
        bl = np.asarray(inputs[f"bl{i}"], np.float32)
        br = np.asarray(inputs[f"br{i}"], np.float32)
        bc = np.asarray(inputs[f"bc{i}"], np.float32)
        f[f"blc{i}_in"] = np.concatenate([bl + bc, br - bc])[None, :]
        f[f"g{i}_in"] = np.asarray(inputs[f"g{i}"], np.float32)[:, None]
        f[f"be{i}_in"] = np.asarray(inputs[f"be{i}"], np.float32)[:, None]
    w5 = np.asarray(inputs["W5"], np.float32)
    w5eff = w5[:224].copy()
    w5eff[192:224] += w5[224:256]
    f["w5_in"] = w5eff
    f["b5_in"] = np.asarray(inputs["b5"], np.float32)[:, None]
    f["g5_in"] = np.asarray(inputs["g5"], np.float32)[:, None]
    f["be5_in"] = np.asarray(inputs["be5"], np.float32)[:, None]
    f["w6_in"] = np.asarray(inputs["W6"], np.float32)
    f["b6_in"] = np.asarray(inputs["b6"], np.float32)[:, None]
    return f


def build_in_maps(inputs, cfg: Cfg, per_dev):
    x = np.asarray(inputs["x"], np.float32)
    batch = np.asarray(inputs["batch"], np.int64)
    folds = _fold_weights(inputs, cfg)
    cnt = np.bincount(batch, minlength=cfg.g).astype(np.float32)
    cnt_rep = np.tile(cnt[None, :], (128, 1))
    iota8 = np.tile(np.arange(128, dtype=np.float32), (128, SG_CHUNKS)).astype(BFNP)
    onesrow = np.ones((1, 128), np.float32)
    in_maps = []
    zeros_cnt = np.zeros_like(cnt_rep)
    for d in range(cfg.ndev):
        pd = per_dev[d]
        xs = np.zeros((cfg.n_pad, 128), np.float32)
        xs[:cfg.n_loc] = x[d * cfg.n_loc:(d + 1) * cfg.n_loc]
        xt = xs.T.astype(BFNP).copy()
        im = dict(xt_in=xt, src16_in=pd["src16"], sel0t_in=pd["sel0t"],
                  dshift_in=pd["dst_shift"], selg_in=pd["selg"],
                  cntrep_in=cnt_rep if d == 0 else zeros_cnt,
                  iota8_in=iota8,
                  onesrow_in=onesrow, **folds)
        in_maps.append(im)
    return in_maps


_CACHE = {}


def _get_program(cfg: Cfg, n_chunks_pad: int):
    key = (cfg.n, cfg.e, cfg.g, cfg.ndev, cfg.cw, n_chunks_pad)
    if key not in _CACHE:
        _CACHE[key] = build_program(cfg, n_chunks_pad)
    return _CACHE[key]


def _maybe_profile():
    """Optional NTFF capture driven by GAT_PROFILE_DIR (self-contained)."""
    import contextlib
    d = os.environ.get("GAT_PROFILE_DIR")
    if not d:
        return contextlib.nullcontext()
    import ctypes
    import glob

    os.makedirs(d, exist_ok=True)
    for f in glob.glob(d + "/*"):
        os.remove(f)
    lib = ctypes.CDLL("/opt/axon/libaxon_pjrt.so")
    lib.axon_start_nrt_profile.argtypes = [ctypes.POINTER(ctypes.c_int64),
                                           ctypes.c_size_t]
    lib.axon_start_nrt_profile.restype = ctypes.c_int64
    lib.axon_stop_nrt_profile.argtypes = [ctypes.c_char_p]
    lib.axon_stop_nrt_profile.restype = ctypes.c_int64

    @contextlib.contextmanager
    def ctx():
        import jax
        jax.devices()
        rc = lib.axon_start_nrt_profile(None, 0)
        if rc != 0:
            raise RuntimeError(f"profile start rc={rc}")
        try:
            yield
        finally:
            lib.axon_stop_nrt_profile(str(d).encode())

    return ctx()


def kernel(**inputs):
    cfg = Cfg()
    per_dev, n_chunks_pad = preprocess(inputs["edge_index"], inputs["batch"], cfg)
    nc = _get_program(cfg, n_chunks_pad)
    in_maps = build_in_maps(inputs, cfg, per_dev)
    with _maybe_profile():
        res = bass_utils.run_bass_kernel_spmd(nc, in_maps,
                                              core_ids=list(range(cfg.ndev)))
    out = np.asarray(res.results[0]["out"])
    return (out[0], out[1])
